# revision 37
# baseline (speedup 1.0000x reference)
"""Trainium2 Bass kernel for nn_DualPathTransformerLayer.

Sharding: data-parallel over batch -- B=8 batch elements, one per NeuronCore.
Each core runs an identical single-core program (SPMD) on its own batch slice;
weights are broadcast. No collectives needed.

Single-core program layout summary:
  - activations kept feature-major ("T" = [channels, positions]) where matmuls
    consume them, natural where needed (values for attention, outputs)
  - all BN affines folded into weights/bias host-side (inference BN)
  - pixel attention: sigmoid(x) = 0.5 + 0.5*tanh(x/2); the 0.5 factors and the
    BN-ret gamma are folded into the value projection; tanh on ACT engine
  - memory attention: softmax without max-subtraction (|logits| <= ~9);
    logits computed transposed [keys, (head, query)] via a block-diagonal
    query operand; denominator rides the AV matmul as an extra ones column
  - matmuls in float32r (1 cyc/row at N>=256) except tiny ones
"""
import numpy as np
import concourse.bass as bass
from concourse import bacc
import concourse.mybir as mybir
import concourse.tile as tile
from concourse.bass_utils import run_bass_kernel_spmd
from concourse.masks import make_identity

# problem dims (hardcoded per contract)
B, L, N, F = 8, 4096, 128, 128
H, DK, DV = 8, 16, 32
TK, TV = H * DK, H * DV          # 128, 256
BOT, FFN = 256, 2048
EPS = 1e-3
INV = float(1.0 / np.sqrt(1.0 + EPS))
P = 128
NCHUNK = 8          # pixel l-chunks of 512
LC = 512            # l-chunk size
MT = (L + N) // P   # 33 m-tiles for memory attention

F32 = mybir.dt.float32
F32R = mybir.dt.float32r
AF = mybir.ActivationFunctionType
ALU = mybir.AluOpType

_cached = {}
DEBUG = False


def _prep_host(w):
    """Fold BN affines into weights/biases. Returns dict of np arrays."""
    d = {}
    g = lambda p: np.asarray(p[0], np.float32) * INV
    be = lambda p: np.asarray(p[1], np.float32)

    def f32(x):
        return np.ascontiguousarray(x, np.float32)

    # ---- pixel conv1 ----
    g1, b1 = g(w["bn_pix1"]), be(w["bn_pix1"])
    W1 = np.asarray(w["W_pix1"], np.float32) * g1[None, :]          # [128,256]
    d["w1"] = f32(W1.reshape(F, 2, P))                              # lhsT [c, mc, m]
    d["b1"] = f32(b1.reshape(2, P).T)                               # [128, 2]

    # ---- pixel qkv ----
    gq, bq = g(w["bn_pix_qkv"]), be(w["bn_pix_qkv"])
    Wq = np.asarray(w["W_pix_qkv"], np.float32)
    Wqk = Wq * gq[None, :]
    bqk = bq.copy()
    # fold pixel-sim gamma into pixel q columns (per head)
    gs_pix = g(w["bn_pix_sim"])        # [H]
    bs_pix = be(w["bn_pix_sim"])       # [H]
    Wq_q = Wqk[:, :TK].copy()
    bq_q = bqk[:TK].copy()
    for h in range(H):
        Wq_q[:, h * DK:(h + 1) * DK] *= gs_pix[h]
        bq_q[h * DK:(h + 1) * DK] *= gs_pix[h]
    # pad q to 32-per-head; slot 16 is the constant-1 channel (bias-in-matmul)
    Wq_pad = np.zeros((BOT, H * 32), np.float32)
    bq_pad = np.zeros(H * 32, np.float32)
    for h in range(H):
        Wq_pad[:, h * 32:h * 32 + DK] = Wq_q[:, h * DK:(h + 1) * DK]
        bq_pad[h * 32:h * 32 + DK] = bq_q[h * DK:(h + 1) * DK]
        bq_pad[h * 32 + DK] = 1.0
    d["wq"] = f32(Wq_pad.reshape(2, P, 2, P).transpose(1, 0, 2, 3))  # [c,kc,mc,m]
    d["bq"] = f32(bq_pad.reshape(2, P).T)                            # [128,2]
    # k compact
    d["wk"] = f32(Wqk[:, TK:2 * TK].reshape(2, P, P).transpose(1, 0, 2))  # [c,kc,m]
    d["bk"] = f32(bqk[TK:2 * TK].reshape(P, 1))
    # v: fold mem-ret gamma (pv feeds only the memory-path v_cat)
    g_mret = g(w["bn_mem_ret"]).reshape(TV)      # [(h,dv)]
    b_mret = be(w["bn_mem_ret"]).reshape(TV)
    Wv_p = Wqk[:, 2 * TK:] * g_mret[None, :]
    bv_p = bqk[2 * TK:] * g_mret
    d["wv_p"] = f32(Wv_p.reshape(2, P, TV).transpose(1, 0, 2))       # [c,kc,n]
    d["bv_p_bc"] = f32(np.tile(bv_p[None, :], (P, 1)))               # [128,256]
    d["b_mret_col"] = f32(b_mret.reshape(2, P).T)                    # [128,2]

    # ---- memory conv1 ----
    gm1, bm1 = g(w["bn_mem1"]), be(w["bn_mem1"])
    Wm1 = np.asarray(w["W_mem1"], np.float32) * gm1[None, :]
    d["wm1"] = f32(Wm1.reshape(F, 2, P))
    d["bm1"] = f32(bm1.reshape(2, P).T)

    # ---- memory qkv ----
    gqm, bqm = g(w["bn_mem_qkv"]), be(w["bn_mem_qkv"])
    Wqm = np.asarray(w["W_mem_qkv"], np.float32) * gqm[None, :]
    bqm_f = bqm.copy()
    gs_mem = g(w["bn_mem_sim"])
    # (mem-sim beta cancels in softmax -- dropped)
    Wq_m = Wqm[:, :TK].copy()
    bq_m = bqm_f[:TK].copy()
    for h in range(H):
        Wq_m[:, h * DK:(h + 1) * DK] *= gs_mem[h]
        bq_m[h * DK:(h + 1) * DK] *= gs_mem[h]
    d["wqm"] = f32(Wq_m.reshape(2, P, P).transpose(1, 0, 2))
    d["bqm"] = f32(bq_m.reshape(P, 1))
    # mk compact (for mem-path k_cat)
    d["wkm"] = f32(Wqm[:, TK:2 * TK].reshape(2, P, P).transpose(1, 0, 2))
    d["bkm"] = f32(bqm_f[TK:2 * TK].reshape(P, 1))
    # mk padded (for pixel QK strips; slot 16 carries pixel-sim beta)
    Wk_m = Wqm[:, TK:2 * TK]
    Wk_pad = np.zeros((BOT, H * 32), np.float32)
    bk_pad = np.zeros(H * 32, np.float32)
    for h in range(H):
        Wk_pad[:, h * 32:h * 32 + DK] = Wk_m[:, h * DK:(h + 1) * DK]
        bk_pad[h * 32:h * 32 + DK] = bqm_f[TK + h * DK:TK + (h + 1) * DK]
        bk_pad[h * 32 + DK] = bs_pix[h]
    d["wkmp"] = f32(Wk_pad.reshape(2, P, 2, P).transpose(1, 0, 2, 3))
    d["bkmp"] = f32(bk_pad.reshape(2, P).T)
    # mv for v_cat (mem-ret gamma folded)
    Wv_m = Wqm[:, 2 * TK:]
    bv_m = bqm_f[2 * TK:]
    Wv_mc = Wv_m * g_mret[None, :]
    bv_mc = bv_m * g_mret
    d["wvm_c"] = f32(Wv_mc.reshape(2, P, TV).transpose(1, 0, 2))
    d["bvm_c_bc"] = f32(np.tile(bv_mc[None, :], (P, 1)))
    # mv for pixel AV (0.5 * pix-ret gamma folded)
    g_pret = g(w["bn_pix_ret"]).reshape(TV)
    b_pret = be(w["bn_pix_ret"]).reshape(TV)
    Wv_mp = Wv_m * (0.5 * g_pret)[None, :]
    bv_mp = bv_m * (0.5 * g_pret)
    d["wvm_p"] = f32(Wv_mp.reshape(2, P, TV).transpose(1, 0, 2))
    d["bvm_p_bc"] = f32(np.tile(bv_mp[None, :], (P, 1)))
    d["b_pret_col"] = f32(b_pret.reshape(2, P).T)                    # [128,2]

    # ---- conv3 ----
    g3, b3 = g(w["bn_pix3"]), be(w["bn_pix3"])
    W3 = np.asarray(w["W_pix3"], np.float32) * g3[None, :]           # [256,128]
    d["w3p"] = f32(W3.reshape(2, P, P).transpose(1, 0, 2))           # [c,kc,m]
    d["b3p"] = f32(b3.reshape(P, 1))
    g3m, b3m = g(w["bn_mem3"]), be(w["bn_mem3"])
    W3m = np.asarray(w["W_mem3"], np.float32) * g3m[None, :]
    d["wm3"] = f32(W3m.reshape(2, P, P).transpose(1, 0, 2))
    d["b3m_col"] = f32(b3m.reshape(P, 1))

    # ---- ffn ----
    gf1, bf1 = g(w["bn_ffn1"]), be(w["bn_ffn1"])
    Wf1 = np.asarray(w["W_ffn1"], np.float32) * gf1[None, :]         # [128,2048]
    d["wf1"] = f32(Wf1.reshape(F, 16, P))                            # lhsT [c,mc,m]
    d["bf1"] = f32(bf1.reshape(16, P).T)                             # [128,16]
    gf2, bf2 = g(w["bn_ffn2"]), be(w["bn_ffn2"])
    Wf2 = np.asarray(w["W_ffn2"], np.float32) * gf2[None, :]         # [2048,128]
    d["wf2"] = f32(Wf2.reshape(16, P, P).transpose(1, 0, 2))         # [c,kc,m]
    d["bf2"] = f32(bf2.reshape(P, 1))

    # concat everything into two blobs ([128, X] each) for 2 big DMAs
    rcols, fcols = [], []
    offs = {}
    for name, shape, dt_ in WEIGHT_SPECS:
        a = d[name].reshape(P, -1)
        tgt = rcols if dt_ == F32R else fcols
        off = sum(x.shape[1] for x in tgt)
        offs[name] = off
        tgt.append(a)
    out = {"wblob_r": np.ascontiguousarray(np.concatenate(rcols, axis=1)),
           "wblob_f": np.ascontiguousarray(np.concatenate(fcols, axis=1))}
    return out


WEIGHT_SPECS = [
    # phase A (memory front) first -- their DMA is staged ahead
    ("wm1", (P, 2, P), F32R), ("bm1", (P, 2), F32),
    ("wqm", (P, 2, P), F32R), ("bqm", (P, 1), F32),
    ("wkm", (P, 2, P), F32R), ("bkm", (P, 1), F32),
    ("wkmp", (P, 2, 2, P), F32R), ("bkmp", (P, 2), F32),
    ("wvm_c", (P, 2, TV), F32R), ("bvm_c_bc", (P, TV), F32),
    ("wvm_p", (P, 2, TV), F32R), ("bvm_p_bc", (P, TV), F32),
    ("b_pret_col", (P, 2), F32), ("b3m_col", (P, 1), F32),
    ("w1", (P, 2, P), F32R), ("b1", (P, 2), F32),
    ("wq", (P, 2, 2, P), F32R), ("bq", (P, 2), F32),
    ("wk", (P, 2, P), F32R), ("bk", (P, 1), F32),
    ("wv_p", (P, 2, TV), F32R), ("bv_p_bc", (P, TV), F32),
    # late weights
    ("b_mret_col", (P, 2), F32),
    ("w3p", (P, 2, P), F32R), ("b3p", (P, 1), F32),
    ("wm3", (P, 2, P), F32R),
    ("wf1", (P, 16, P), F32R), ("bf1", (P, 16), F32),
    ("wf2", (P, 16, P), F32R), ("bf2", (P, 1), F32),
]
A_WEIGHTS = ["wm1", "bm1", "wqm", "bqm", "wkm", "bkm", "wkmp", "bkmp",
             "wvm_c", "bvm_c_bc", "wvm_p", "bvm_p_bc", "b_pret_col", "b3m_col",
             "w1", "b1", "wq", "bq", "wk", "bk", "wv_p", "bv_p_bc"]




def _blob_layout():
    ro, fo = {}, {}
    rc = fc = 0
    for name, shape, dt_ in WEIGHT_SPECS:
        ncol = int(np.prod(shape[1:]))
        if dt_ == F32R:
            ro[name] = (rc, ncol)
            rc += ncol
        else:
            fo[name] = (fc, ncol)
            fc += ncol
    return ro, rc, fo, fc


def _build_program():
    nc = bacc.Bacc("TRN2", target_bir_lowering=False)
    xp_d = nc.dram_tensor("xp", (L, F), F32, kind="ExternalInput")
    xm_d = nc.dram_tensor("xm", (N, F), F32, kind="ExternalInput")
    ro, rc, fo, fc = _blob_layout()
    wd = {
        "wblob_r": nc.dram_tensor("wblob_r", (P, rc), F32R, kind="ExternalInput"),
        "wblob_f": nc.dram_tensor("wblob_f", (P, fc), F32, kind="ExternalInput"),
    }
    ypix_d = nc.dram_tensor("ypix", (L, F), F32, kind="ExternalOutput")
    ymem_d = nc.dram_tensor("ymem", (N, F), F32, kind="ExternalOutput")
    dbg = {}
    if DEBUG:
        for nm, shape in (("d_ret", (P, 2, P)), ("d_dsb", (P, 512)),
                          ("d_mo", (P, P)), ("d_ffn", (P, 16, P)),
                          ("d_av", (2, P, LC)), ("d_m1", (P, 2, P))):
            dbg[nm] = nc.dram_tensor(nm, shape, F32, kind="ExternalOutput")

    with tile.TileContext(nc) as tc:
        _emit(nc, tc, xp_d, xm_d, wd, ypix_d, ymem_d, dbg)
    nc.finalize()
    return nc


def _emit(nc, tc, xp_d, xm_d, wd, ypix_d, ymem_d, dbg=None):
    from contextlib import ExitStack
    ctx = ExitStack()
    with ctx:
        const = ctx.enter_context(tc.tile_pool(name="const", bufs=1))
        persist = ctx.enter_context(tc.tile_pool(name="persist", bufs=1))
        trans = ctx.enter_context(tc.tile_pool(name="trans", bufs=2))
        trans3 = ctx.enter_context(tc.tile_pool(name="trans3", bufs=2))
        deep3 = ctx.enter_context(tc.tile_pool(name="deep3", bufs=3))
        mm_ps = ctx.enter_context(tc.tile_pool(name="mm_ps", bufs=3, space="PSUM"))
        tin_ps = ctx.enter_context(tc.tile_pool(name="tin_ps", bufs=1, space="PSUM"))
        qk_ps = ctx.enter_context(tc.tile_pool(name="qk_ps", bufs=2, space="PSUM"))

        # ---------------- constants / weights in SBUF ----------------
        ro, rc, fo, fc = _blob_layout()
        blob_r = const.tile([P, rc], F32R, tag="blob_r", name="blob_r")
        blob_f = const.tile([P, fc], F32, tag="blob_f", name="blob_f")
        # inputs + first pixel chunks first (the DMA pipe is a FIFO), then
        # weights staged by first use
        xm_nat0 = persist.tile([P, P], F32, tag="xm_nat0", name="xm_nat0")
        nc.sync.dma_start(xm_nat0[:], xm_d[:])
        xp_pre = persist.tile([P, 8, P], F32, tag="xp_pre", name="xp_pre")
        nc.sync.dma_start(xp_pre[:],
                          xp_d[0:1024, :].rearrange("(t p) c -> p t c", p=P))
        actwarm = const.tile([P, 2], F32, tag="actwarm", name="actwarm")
        r0_ = ro["wm1"][0] + ro["wm1"][1]
        f0_ = fo["bm1"][0] + fo["bm1"][1]
        ra = max(off + n_ for off, n_ in (ro[k] for k in A_WEIGHTS if k in ro))
        fa = max(off + n_ for off, n_ in (fo[k] for k in A_WEIGHTS if k in fo))
        nc.sync.dma_start(blob_r[:, :r0_], wd["wblob_r"][:, :r0_])
        nc.sync.dma_start(blob_f[:, :f0_], wd["wblob_f"][:, :f0_])
        nc.sync.dma_start(blob_r[:, r0_:ra], wd["wblob_r"][:, r0_:ra])
        nc.sync.dma_start(blob_f[:, f0_:fa], wd["wblob_f"][:, f0_:fa])
        nc.sync.dma_start(blob_r[:, ra:], wd["wblob_r"][:, ra:])
        nc.sync.dma_start(blob_f[:, fa:], wd["wblob_f"][:, fa:])
        W = {}
        for name, shape, dt_ in WEIGHT_SPECS:
            if dt_ == F32R:
                off, ncol = ro[name]
                ap = blob_r[:, off:off + ncol]
            else:
                off, ncol = fo[name]
                ap = blob_f[:, off:off + ncol]
            if len(shape) == 3:
                ap = ap.rearrange("p (a b) -> p a b", b=shape[2])
            elif len(shape) == 4:
                ap = ap.rearrange("p (a b c) -> p a b c", b=shape[2], c=shape[3])
            W[name] = ap
        ident = const.tile([P, P], F32, tag="ident")
        make_identity(nc, ident[:])
        ones_r = const.tile([P, 2], F32R, tag="ones_r")
        nc.vector.memset(ones_r[:].bitcast(F32), 1.0)
        ones64 = const.tile([P, 64], F32, tag="ones64")
        nc.vector.memset(ones64[:], 1.0)
        nc.scalar.activation(actwarm[:], ones64[:, 0:2], AF.Tanh)

        # ---------------- persistent buffers ----------------
        xpT = persist.tile([P, L], F32, tag="xpT")             # 16 KB/part
        kcatT = persist.tile([P, L + N], F32R, tag="kcatT")    # 16.5 KB
        vaug = persist.tile([P, MT, 260], F32R, tag="vaug")    # 33.4 KB
        mq_bd = persist.tile([P, H * P], F32R, tag="mq_bd")    # 4 KB
        mvpx_pad = persist.tile([P, 2, 4, P], F32R, tag="mvpx_pad")  # 4 KB
        biaspret = persist.tile([P, 2], F32, tag="biaspret")
        xm_b3T = persist.tile([P, P], F32, tag="xm_b3T")
        mkpT = persist.tile([P, 2, P], F32R, tag="mkpT")
        mvpx = persist.tile([P, TV], F32R, tag="mvpx")
        retT = persist.tile([P, 2, P], F32R, tag="retT")

        # ones columns of vaug (slot 64 of each 65-wide pair block)
        nc.vector.memset(
            vaug[:].rearrange("p t (pr c) -> p t pr c", c=65)[:, :, :, 64:65]
            .bitcast(F32), 1.0)

        # ================= PHASE A: memory front =================
        ps = mm_ps.tile([P, LC], F32, tag="mm")
        nc.tensor.transpose(ps[:, 0:P], xm_nat0[:], ident[:])
        xmT = trans.tile([P, P], F32R, tag="xmT")
        nc.vector.tensor_copy(xmT[:], ps[:, 0:P])
        nc.vector.tensor_scalar(xm_b3T[:], ps[:, 0:P], W["b3m_col"][:], None,
                                ALU.add)

        # M1T feature-major [2][128, 128]
        m1T = persist.tile([P, 2, P], F32R, tag="m1T")
        for mc in range(2):
            pm = mm_ps.tile([P, LC], F32, tag="mm")
            nc.tensor.matmul(pm[:, 0:P], W["wm1"][:, mc, :], xmT[:],
                             start=True, stop=True)
            nc.vector.tensor_scalar(m1T[:, mc, :], pm[:, 0:P],
                                    W["bm1"][:, mc:mc + 1], 0.0, ALU.add, ALU.max)

        # mqT compact -> mq_bd blockdiag
        pm = mm_ps.tile([P, LC], F32, tag="mm")
        for kc in range(2):
            nc.tensor.matmul(pm[:, 0:P], W["wqm"][:, kc, :], m1T[:, kc, :],
                             start=(kc == 0), stop=(kc == 1))
        mqT = trans.tile([P, P], F32R, tag="mqT")
        nc.vector.tensor_scalar(mqT[:], pm[:, 0:P], W["bqm"][:], None, ALU.add)
        nc.vector.memset(mq_bd[:].bitcast(F32), 0.0)
        for h in range(H):
            nc.sync.dma_start(mq_bd[h * DK:(h + 1) * DK, h * P:(h + 1) * P],
                              mqT[h * DK:(h + 1) * DK, :])

        # mkT compact -> kcatT tail
        pm = mm_ps.tile([P, LC], F32, tag="mm")
        for kc in range(2):
            nc.tensor.matmul(pm[:, 0:P], W["wkm"][:, kc, :], m1T[:, kc, :],
                             start=(kc == 0), stop=(kc == 1))
        nc.vector.tensor_scalar(kcatT[:, L:L + N], pm[:, 0:P], W["bkm"][:],
                                None, ALU.add)

        # mk padded (pixel QK lhsT)
        for mc in range(2):
            pm = mm_ps.tile([P, LC], F32, tag="mm")
            for kc in range(2):
                nc.tensor.matmul(pm[:, 0:P], W["wkmp"][:, kc, mc, :],
                                 m1T[:, kc, :], start=(kc == 0), stop=(kc == 1))
            nc.vector.tensor_scalar(mkpT[:, mc, :], pm[:, 0:P],
                                    W["bkmp"][:, mc:mc + 1], None, ALU.add)

        # mv for v_cat -> vaug chunk 32
        pm = mm_ps.tile([P, LC], F32, tag="mm")
        for kc in range(2):
            nc.tensor.matmul(pm[:, 0:TV], m1T[:, kc, :], W["wvm_c"][:, kc, :],
                             start=(kc == 0), stop=(kc == 1))
        nc.vector.tensor_tensor(
            vaug[:, 32].rearrange("p (pr c) -> p pr c", c=65)[:, :, 0:64],
            pm[:, 0:TV].rearrange("p (pr c) -> p pr c", c=64),
            W["bvm_c_bc"][:].rearrange("p (pr c) -> p pr c", c=64), ALU.add)

        # mv for pixel AV (scaled); plus zero-padded per-head variant
        pm = mm_ps.tile([P, LC], F32, tag="mm")
        for kc in range(2):
            nc.tensor.matmul(pm[:, 0:TV], m1T[:, kc, :], W["wvm_p"][:, kc, :],
                             start=(kc == 0), stop=(kc == 1))
        nc.vector.tensor_tensor(mvpx[:], pm[:, 0:TV], W["bvm_p_bc"][:], ALU.add)
        nc.vector.memset(mvpx_pad[:].bitcast(F32), 0.0)
        for h in range(H):
            g_, i_ = divmod(h, 4)
            nc.vector.tensor_copy(
                mvpx_pad[:, g_, i_, 32 * i_:32 * i_ + 32],
                mvpx[:, 32 * h:32 * h + 32])

        # colsum of mvpx (pixel AV bias) + b_pret
        pm = mm_ps.tile([P, LC], F32, tag="mm")
        for c_ in range(2):
            nc.tensor.matmul(pm[:, 2 * c_:2 * c_ + 2],
                             mvpx[:, P * c_:P * (c_ + 1)],
                             ones_r[:], start=True, stop=True)
        nc.vector.tensor_tensor(biaspret[:], pm[:, 0:4:2], W["b_pret_col"][:],
                                ALU.add)

        # ================= PHASE B: pixel pipeline =================
        # Software-pipelined emission: front(c+1) is emitted before tail(c) so
        # the scheduler can fill attention-phase gaps with next-chunk work.
        def _front(c):
            l0 = c * LC
            pst = tin_ps.tile([P, LC], F32, tag="tin", name="pst")
            for j in range(4):
                if c < 2:
                    xnat = xp_pre[:, 4 * c + j, :]
                else:
                    xt_ = trans3.tile([P, P], F32, tag="xnat", name="xnat")
                    nc.sync.dma_start(xt_[:], xp_d[l0 + P * j:l0 + P * (j + 1), :])
                    xnat = xt_[:]
                nc.tensor.transpose(pst[:, P * j:P * (j + 1)], xnat, ident[:])
            nc.vector.tensor_copy(xpT[:, l0:l0 + LC], pst[:])
            xpTr = trans.tile([P, LC], F32R, tag="xpTr", name="xpTr")
            nc.vector.tensor_copy(xpTr[:], xpT[:, l0:l0 + LC])

            # conv1 -> P1T [2][128, 512]
            p1T = deep3.tile([P, 2, LC], F32R, tag="p1T", name="p1T")
            for mc in range(2):
                pm = tin_ps.tile([P, LC], F32, tag="tin", name="pm")
                nc.tensor.matmul(pm[:], W["w1"][:, mc, :], xpTr[:],
                                 start=True, stop=True)
                nc.scalar.activation(p1T[:, mc, :], pm[:], AF.Relu,
                                     bias=W["b1"][:, mc:mc + 1])

            # qkv projections
            pqTp = deep3.tile([P, 2, LC], F32R, tag="pqTp", name="pqTp")
            for mc in range(2):
                pm = mm_ps.tile([P, LC], F32, tag="mm", name="pm")
                for kc in range(2):
                    nc.tensor.matmul(pm[:], W["wq"][:, kc, mc, :], p1T[:, kc, :],
                                     start=(kc == 0), stop=(kc == 1))
                nc.vector.tensor_scalar(pqTp[:, mc, :], pm[:],
                                        W["bq"][:, mc:mc + 1], None, ALU.add)
            pm = mm_ps.tile([P, LC], F32, tag="mm", name="pm")
            for kc in range(2):
                nc.tensor.matmul(pm[:], W["wk"][:, kc, :], p1T[:, kc, :],
                                 start=(kc == 0), stop=(kc == 1))
            nc.vector.tensor_scalar(kcatT[:, l0:l0 + LC], pm[:], W["bk"][:],
                                    None, ALU.add)
            for lt in range(4):
                pm = mm_ps.tile([P, LC], F32, tag="mm", name="pm")
                for kc in range(2):
                    nc.tensor.matmul(pm[:, 0:TV],
                                     p1T[:, kc, P * lt:P * (lt + 1)],
                                     W["wv_p"][:, kc, :],
                                     start=(kc == 0), stop=(kc == 1))
                nc.vector.tensor_tensor(
                    vaug[:, 4 * c + lt].rearrange("p (pr x) -> p pr x", x=65)
                    [:, :, 0:64],
                    pm[:, 0:TV].rearrange("p (pr x) -> p pr x", x=64),
                    W["bv_p_bc"][:].rearrange("p (pr x) -> p pr x", x=64),
                    ALU.add)
            return pqTp

        def _tail(c, pqTp):
            l0 = c * LC
            # pixel QK (row-packed strips, 2 heads per round) + tanh,
            # then AV per 4-head group (4 accumulating zero-padded MMs)
            pretT = trans.tile([P, 2, LC], F32R, tag="pretT", name="pretT")
            for g_ in range(2):
                pattn = trans.tile([P, 4, LC], F32R, tag="pattn", name="pattn")
                for r in range(2):
                    pq_ = qk_ps.tile([P, 2 * LC], F32, tag="qk", name="pq_")
                    for i in range(2):
                        h = 4 * g_ + 2 * r + i
                        pos = 32 * (h % 4)
                        nc.tensor.matmul(pq_[:, LC * i:LC * (i + 1)],
                                         mkpT[pos:pos + 32, h // 4, :],
                                         pqTp[pos:pos + 32, h // 4, :],
                                         start=True, stop=True,
                                         tile_position=(pos, 0))
                    nc.scalar.activation(
                        pattn[:, 2 * r:2 * r + 2, :]
                        .rearrange("p a b -> p (a b)"),
                        pq_[:], AF.Tanh, scale=0.5)
                pm = mm_ps.tile([P, LC], F32, tag="mm", name="pm")
                for i in range(4):
                    nc.tensor.matmul(pm[:], mvpx_pad[:, g_, i, :],
                                     pattn[:, i, :],
                                     start=(i == 0), stop=(i == 3))
                nc.vector.tensor_scalar(pretT[:, g_, :], pm[:],
                                        biaspret[:, g_:g_ + 1], 0.0,
                                        ALU.add, ALU.max)

            # conv3 + residual + relu (feature-major), then transpose out
            pm = mm_ps.tile([P, LC], F32, tag="mm", name="pm")
            for kc in range(2):
                nc.tensor.matmul(pm[:], W["w3p"][:, kc, :], pretT[:, kc, :],
                                 start=(kc == 0), stop=(kc == 1))
            poutT = trans.tile([P, LC], F32, tag="poutT", name="poutT")
            nc.vector.tensor_tensor(poutT[:], pm[:], xpT[:, l0:l0 + LC], ALU.add)
            nc.vector.tensor_scalar(poutT[:], poutT[:], W["b3p"][:], 0.0,
                                    ALU.add, ALU.max)
            pst2 = mm_ps.tile([P, LC], F32, tag="mm", name="pst2")
            for j in range(4):
                nc.tensor.transpose(pst2[:, P * j:P * (j + 1)],
                                    poutT[:, P * j:P * (j + 1)], ident[:])
            pout = trans.tile([P, LC], F32, tag="poutT", name="pout")
            nc.vector.tensor_copy(pout[:], pst2[:])
            for j in range(4):
                nc.sync.dma_start(ypix_d[l0 + P * j:l0 + P * (j + 1), :],
                                  pout[:, P * j:P * (j + 1)])

        pend = {}
        for c in range(NCHUNK + 1):
            if c < NCHUNK:
                pend[c] = _front(c)
            if c >= 1:
                _tail(c - 1, pend.pop(c - 1))

        # ================= PHASE C: memory attention =================
        avp = [mm_ps.tile([P, LC], F32, tag="mm", name=f"av{q}") for q in range(2)]
        for t in range(MT):
            pqm = qk_ps.tile([P, 2 * LC], F32, tag="qk")
            for u in range(2):
                nc.tensor.matmul(pqm[:, LC * u:LC * (u + 1)],
                                 kcatT[:, P * t:P * (t + 1)],
                                 mq_bd[:, LC * u:LC * (u + 1)],
                                 start=True, stop=True)
            probs = trans3.tile([P, 2 * LC], F32R, tag="probs")
            nc.scalar.activation(probs[:], pqm[:], AF.Exp)
            for pr in range(4):
                nc.tensor.matmul(
                    avp[pr // 2][0:65, 256 * (pr % 2):256 * (pr % 2) + 256],
                    vaug[:, t, 65 * pr:65 * pr + 65],
                    probs[:, 256 * pr:256 * (pr + 1)],
                    start=(t == 0), stop=(t == MT - 1))

        # ================= PHASE D: memory tail =================
        # denominator rows: one pair at a time through a small row-64 buffer
        dsb = persist.tile([P, 1024], F32, tag="dsb")
        for pr in range(4):
            q, s_ = divmod(pr, 2)
            col = 256 * pr
            nc.vector.tensor_copy(dsb[64:65, col:col + 256],
                                  avp[q][64:65, 256 * s_:256 * (s_ + 1)])
            psd = qk_ps.tile([P, 2 * LC], F32, tag="qk")
            nc.tensor.matmul(psd[0:64, 0:256], ones64[64:65, :],
                             dsb[64:65, col:col + 256],
                             start=True, stop=True, tile_position=(64, 0))
            recip = trans3.tile([64, 256], F32, tag="recip")
            nc.vector.reciprocal(recip[:], psd[0:64, 0:256])
            ro = 64 * s_
            nc.vector.tensor_tensor(retT[ro:ro + 32, q, :],
                                    avp[q][0:32, 256 * s_:256 * s_ + P],
                                    recip[0:32, 0:P], ALU.mult)
            nc.vector.tensor_tensor(retT[ro + 32:ro + 64, q, :],
                                    avp[q][32:64, 256 * s_ + P:256 * (s_ + 1)],
                                    recip[32:64, P:2 * P], ALU.mult)
        for q in range(2):
            nc.vector.tensor_scalar(retT[:, q, :], retT[:, q, :],
                                    W["b_mret_col"][:, q:q + 1], 0.0,
                                    ALU.add, ALU.max)

        if dbg:
            for q in range(2):
                dd = trans.tile([P, LC], F32, tag="poutT", name="dd")[:, 0:P]
                nc.vector.tensor_copy(dd[:], retT[:, q, :])
                nc.sync.dma_start(dbg["d_ret"][:, q, :], dd[:])
                da = trans.tile([P, LC], F32, tag="poutT", name="da")
                nc.vector.tensor_copy(da[:], avp[q][:])
                nc.sync.dma_start(dbg["d_av"][q], da[:])
                dm1 = trans.tile([P, LC], F32, tag="poutT", name="dm1")[:, 0:P]
                nc.vector.tensor_copy(dm1[:], m1T[:, q, :])
                nc.sync.dma_start(dbg["d_m1"][:, q, :], dm1[:])
            nc.sync.dma_start(dbg["d_dsb"][:, 0:256], dsb[:, 0:256])

        # mem conv3 feature-major (+ residual + relu) -> moT directly
        pm = mm_ps.tile([P, LC], F32, tag="mm")
        for kc in range(2):
            nc.tensor.matmul(pm[:, 0:P], W["wm3"][:, kc, :], retT[:, kc, :],
                             start=(kc == 0), stop=(kc == 1))
        moT = trans.tile([P, P], F32, tag="moT")
        nc.vector.tensor_tensor(moT[:], pm[:, 0:P], xm_b3T[:], ALU.add)
        nc.vector.tensor_scalar(moT[:], moT[:], 0.0, None, ALU.max)
        moTr = trans.tile([P, P], F32R, tag="moTr")
        nc.vector.tensor_copy(moTr[:], moT[:])

        if dbg:
            dmo = trans.tile([P, LC], F32, tag="poutT", name="dmo")[:, 0:P]
            nc.vector.tensor_copy(dmo[:], mo[:])
            nc.sync.dma_start(dbg["d_mo"][:], dmo[:])

        # ffn1 -> ffnT [128, 16, 128]
        ffnT = persist.tile([P, 16, P], F32R, tag="ffnT")
        for g_ in range(4):
            pm = mm_ps.tile([P, LC], F32, tag="mm")
            for s_ in range(4):
                mc = 4 * g_ + s_
                nc.tensor.matmul(pm[:, P * s_:P * (s_ + 1)],
                                 W["wf1"][:, mc, :], moTr[:],
                                 start=True, stop=True)
            for s_ in range(4):
                mc = 4 * g_ + s_
                nc.scalar.activation(ffnT[:, mc, :],
                                     pm[:, P * s_:P * (s_ + 1)], AF.Relu,
                                     bias=W["bf1"][:, mc:mc + 1])
        if dbg:
            for mc in range(16):
                df = trans.tile([P, LC], F32, tag="poutT", name="df")[:, 0:P]
                nc.vector.tensor_copy(df[:], ffnT[:, mc, :])
                nc.sync.dma_start(dbg["d_ffn"][:, mc, :], df[:])

        # ffn2 (+ residual + relu) -> transpose -> ymem
        # two independent accumulators so the chain overlaps ffn1 production
        pma = mm_ps.tile([P, LC], F32, tag="mm", name="pma")
        pmb = mm_ps.tile([P, LC], F32, tag="mm", name="pmb")
        for kc in range(8):
            nc.tensor.matmul(pma[:, 0:P], W["wf2"][:, kc, :], ffnT[:, kc, :],
                             start=(kc == 0), stop=(kc == 7))
        for kc in range(8, 16):
            nc.tensor.matmul(pmb[:, 0:P], W["wf2"][:, kc, :], ffnT[:, kc, :],
                             start=(kc == 8), stop=(kc == 15))
        mo2T = trans.tile([P, P], F32, tag="mo2T")
        nc.vector.tensor_tensor(mo2T[:], pma[:, 0:P], moT[:], ALU.add)
        nc.vector.tensor_tensor(mo2T[:], pmb[:, 0:P], mo2T[:], ALU.add)
        nc.vector.tensor_scalar(mo2T[:], mo2T[:], W["bf2"][:], 0.0,
                                ALU.add, ALU.max)
        pst = mm_ps.tile([P, LC], F32, tag="mm")
        nc.tensor.transpose(pst[:, 0:P], mo2T[:], ident[:])
        mo2 = trans.tile([P, P], F32, tag="mo2")
        nc.vector.tensor_copy(mo2[:], pst[:, 0:P])
        nc.sync.dma_start(ymem_d[:], mo2[:])


def kernel(**inputs):
    if "nc" not in _cached:
        _cached["nc"] = _build_program()
    nc = _cached["nc"]
    d = _prep_host(inputs)
    pix = np.ascontiguousarray(np.asarray(inputs["pixel_input"], np.float32))
    mem = np.ascontiguousarray(np.asarray(inputs["memory_input"], np.float32))
    in_maps = []
    for b in range(B):
        m = {"xp": pix[b], "xm": mem[b]}
        m.update(d)
        in_maps.append(m)
    res = run_bass_kernel_spmd(nc, in_maps, core_ids=list(range(B)))
    pix_out = np.stack([res.results[b]["ypix"] for b in range(B)])
    mem_out = np.stack([res.results[b]["ymem"] for b in range(B)])
    return pix_out, mem_out


# revision 39
# speedup vs baseline: 1.0151x; 1.0151x over previous
"""Trainium2 Bass kernel for nn_DualPathTransformerLayer.

Sharding: data-parallel over batch -- B=8 batch elements, one per NeuronCore.
Each core runs an identical single-core program (SPMD) on its own batch slice;
weights are broadcast. No collectives needed.

Single-core program layout summary:
  - activations kept feature-major ("T" = [channels, positions]) where matmuls
    consume them, natural where needed (values for attention, outputs)
  - all BN affines folded into weights/bias host-side (inference BN)
  - pixel attention: sigmoid(x) = 0.5 + 0.5*tanh(x/2); the 0.5 factors and the
    BN-ret gamma are folded into the value projection; tanh on ACT engine
  - memory attention: softmax without max-subtraction (|logits| <= ~9);
    logits computed transposed [keys, (head, query)] via a block-diagonal
    query operand; denominator rides the AV matmul as an extra ones column
  - matmuls in float32r (1 cyc/row at N>=256) except tiny ones
"""
import numpy as np
import concourse.bass as bass
from concourse import bacc
import concourse.mybir as mybir
import concourse.tile as tile
from concourse.bass_utils import run_bass_kernel_spmd
from concourse.masks import make_identity

# problem dims (hardcoded per contract)
B, L, N, F = 8, 4096, 128, 128
H, DK, DV = 8, 16, 32
TK, TV = H * DK, H * DV          # 128, 256
BOT, FFN = 256, 2048
EPS = 1e-3
INV = float(1.0 / np.sqrt(1.0 + EPS))
P = 128
NCHUNK = 8          # pixel l-chunks of 512
LC = 512            # l-chunk size
MT = (L + N) // P   # 33 m-tiles for memory attention

F32 = mybir.dt.float32
F32R = mybir.dt.float32r
AF = mybir.ActivationFunctionType
ALU = mybir.AluOpType

_cached = {}
DEBUG = False


def _prep_host(w):
    """Fold BN affines into weights/biases. Returns dict of np arrays."""
    d = {}
    g = lambda p: np.asarray(p[0], np.float32) * INV
    be = lambda p: np.asarray(p[1], np.float32)

    def f32(x):
        return np.ascontiguousarray(x, np.float32)

    # ---- pixel conv1 ----
    g1, b1 = g(w["bn_pix1"]), be(w["bn_pix1"])
    W1 = np.asarray(w["W_pix1"], np.float32) * g1[None, :]          # [128,256]
    d["w1"] = f32(W1.reshape(F, 2, P))                              # lhsT [c, mc, m]
    d["b1"] = f32(b1.reshape(2, P).T)                               # [128, 2]

    # ---- pixel qkv ----
    gq, bq = g(w["bn_pix_qkv"]), be(w["bn_pix_qkv"])
    Wq = np.asarray(w["W_pix_qkv"], np.float32)
    Wqk = Wq * gq[None, :]
    bqk = bq.copy()
    # fold pixel-sim gamma into pixel q columns (per head)
    gs_pix = g(w["bn_pix_sim"])        # [H]
    bs_pix = be(w["bn_pix_sim"])       # [H]
    Wq_q = Wqk[:, :TK].copy()
    bq_q = bqk[:TK].copy()
    for h in range(H):
        Wq_q[:, h * DK:(h + 1) * DK] *= gs_pix[h]
        bq_q[h * DK:(h + 1) * DK] *= gs_pix[h]
    # pad q to 32-per-head; slot 16 is the constant-1 channel (bias-in-matmul)
    Wq_pad = np.zeros((BOT, H * 32), np.float32)
    bq_pad = np.zeros(H * 32, np.float32)
    for h in range(H):
        Wq_pad[:, h * 32:h * 32 + DK] = Wq_q[:, h * DK:(h + 1) * DK]
        bq_pad[h * 32:h * 32 + DK] = bq_q[h * DK:(h + 1) * DK]
        bq_pad[h * 32 + DK] = 1.0
    d["wq"] = f32(Wq_pad.reshape(2, P, 2, P).transpose(1, 0, 2, 3))  # [c,kc,mc,m]
    d["bq"] = f32(bq_pad.reshape(2, P).T)                            # [128,2]
    # k compact
    d["wk"] = f32(Wqk[:, TK:2 * TK].reshape(2, P, P).transpose(1, 0, 2))  # [c,kc,m]
    d["bk"] = f32(bqk[TK:2 * TK].reshape(P, 1))
    # v: fold mem-ret gamma (pv feeds only the memory-path v_cat)
    g_mret = g(w["bn_mem_ret"]).reshape(TV)      # [(h,dv)]
    b_mret = be(w["bn_mem_ret"]).reshape(TV)
    Wv_p = Wqk[:, 2 * TK:] * g_mret[None, :]
    bv_p = bqk[2 * TK:] * g_mret
    d["wv_p"] = f32(Wv_p.reshape(2, P, TV).transpose(1, 0, 2))       # [c,kc,n]
    d["bv_p_bc"] = f32(np.tile(bv_p[None, :], (P, 1)))               # [128,256]
    d["b_mret_col"] = f32(b_mret.reshape(2, P).T)                    # [128,2]

    # ---- memory conv1 ----
    gm1, bm1 = g(w["bn_mem1"]), be(w["bn_mem1"])
    Wm1 = np.asarray(w["W_mem1"], np.float32) * gm1[None, :]
    d["wm1"] = f32(Wm1.reshape(F, 2, P))
    d["bm1"] = f32(bm1.reshape(2, P).T)

    # ---- memory qkv ----
    gqm, bqm = g(w["bn_mem_qkv"]), be(w["bn_mem_qkv"])
    Wqm = np.asarray(w["W_mem_qkv"], np.float32) * gqm[None, :]
    bqm_f = bqm.copy()
    gs_mem = g(w["bn_mem_sim"])
    # (mem-sim beta cancels in softmax -- dropped)
    Wq_m = Wqm[:, :TK].copy()
    bq_m = bqm_f[:TK].copy()
    for h in range(H):
        Wq_m[:, h * DK:(h + 1) * DK] *= gs_mem[h]
        bq_m[h * DK:(h + 1) * DK] *= gs_mem[h]
    d["wqm"] = f32(Wq_m.reshape(2, P, P).transpose(1, 0, 2))
    d["bqm"] = f32(bq_m.reshape(P, 1))
    # mk compact (for mem-path k_cat)
    d["wkm"] = f32(Wqm[:, TK:2 * TK].reshape(2, P, P).transpose(1, 0, 2))
    d["bkm"] = f32(bqm_f[TK:2 * TK].reshape(P, 1))
    # mk padded (for pixel QK strips; slot 16 carries pixel-sim beta)
    Wk_m = Wqm[:, TK:2 * TK]
    Wk_pad = np.zeros((BOT, H * 32), np.float32)
    bk_pad = np.zeros(H * 32, np.float32)
    for h in range(H):
        Wk_pad[:, h * 32:h * 32 + DK] = Wk_m[:, h * DK:(h + 1) * DK]
        bk_pad[h * 32:h * 32 + DK] = bqm_f[TK + h * DK:TK + (h + 1) * DK]
        bk_pad[h * 32 + DK] = bs_pix[h]
    d["wkmp"] = f32(Wk_pad.reshape(2, P, 2, P).transpose(1, 0, 2, 3))
    d["bkmp"] = f32(bk_pad.reshape(2, P).T)
    # mv for v_cat (mem-ret gamma folded)
    Wv_m = Wqm[:, 2 * TK:]
    bv_m = bqm_f[2 * TK:]
    Wv_mc = Wv_m * g_mret[None, :]
    bv_mc = bv_m * g_mret
    d["wvm_c"] = f32(Wv_mc.reshape(2, P, TV).transpose(1, 0, 2))
    d["bvm_c_bc"] = f32(np.tile(bv_mc[None, :], (P, 1)))
    # mv for pixel AV (0.5 * pix-ret gamma folded)
    g_pret = g(w["bn_pix_ret"]).reshape(TV)
    b_pret = be(w["bn_pix_ret"]).reshape(TV)
    Wv_mp = Wv_m * (0.5 * g_pret)[None, :]
    bv_mp = bv_m * (0.5 * g_pret)
    d["wvm_p"] = f32(Wv_mp.reshape(2, P, TV).transpose(1, 0, 2))
    d["bvm_p_bc"] = f32(np.tile(bv_mp[None, :], (P, 1)))
    d["b_pret_col"] = f32(b_pret.reshape(2, P).T)                    # [128,2]

    # ---- conv3 ----
    g3, b3 = g(w["bn_pix3"]), be(w["bn_pix3"])
    W3 = np.asarray(w["W_pix3"], np.float32) * g3[None, :]           # [256,128]
    d["w3p"] = f32(W3.reshape(2, P, P).transpose(1, 0, 2))           # [c,kc,m]
    d["b3p"] = f32(b3.reshape(P, 1))
    g3m, b3m = g(w["bn_mem3"]), be(w["bn_mem3"])
    W3m = np.asarray(w["W_mem3"], np.float32) * g3m[None, :]
    d["wm3"] = f32(W3m.reshape(2, P, P).transpose(1, 0, 2))
    d["b3m_col"] = f32(b3m.reshape(P, 1))

    # ---- ffn ----
    gf1, bf1 = g(w["bn_ffn1"]), be(w["bn_ffn1"])
    Wf1 = np.asarray(w["W_ffn1"], np.float32) * gf1[None, :]         # [128,2048]
    d["wf1"] = f32(Wf1.reshape(F, 16, P))                            # lhsT [c,mc,m]
    d["bf1"] = f32(bf1.reshape(16, P).T)                             # [128,16]
    gf2, bf2 = g(w["bn_ffn2"]), be(w["bn_ffn2"])
    Wf2 = np.asarray(w["W_ffn2"], np.float32) * gf2[None, :]         # [2048,128]
    d["wf2"] = f32(Wf2.reshape(16, P, P).transpose(1, 0, 2))         # [c,kc,m]
    d["bf2"] = f32(bf2.reshape(P, 1))

    # concat everything into two blobs ([128, X] each) for 2 big DMAs
    rcols, fcols = [], []
    offs = {}
    for name, shape, dt_ in WEIGHT_SPECS:
        a = d[name].reshape(P, -1)
        tgt = rcols if dt_ == F32R else fcols
        off = sum(x.shape[1] for x in tgt)
        offs[name] = off
        tgt.append(a)
    out = {"wblob_r": np.ascontiguousarray(np.concatenate(rcols, axis=1)),
           "wblob_f": np.ascontiguousarray(np.concatenate(fcols, axis=1))}
    return out


WEIGHT_SPECS = [
    # phase A (memory front) first -- their DMA is staged ahead
    ("wm1", (P, 2, P), F32R), ("bm1", (P, 2), F32),
    ("wqm", (P, 2, P), F32R), ("bqm", (P, 1), F32),
    ("wkm", (P, 2, P), F32R), ("bkm", (P, 1), F32),
    ("wkmp", (P, 2, 2, P), F32R), ("bkmp", (P, 2), F32),
    ("wvm_c", (P, 2, TV), F32R), ("bvm_c_bc", (P, TV), F32),
    ("wvm_p", (P, 2, TV), F32R), ("bvm_p_bc", (P, TV), F32),
    ("b_pret_col", (P, 2), F32), ("b3m_col", (P, 1), F32),
    ("w1", (P, 2, P), F32R), ("b1", (P, 2), F32),
    ("wq", (P, 2, 2, P), F32R), ("bq", (P, 2), F32),
    ("wk", (P, 2, P), F32R), ("bk", (P, 1), F32),
    ("wv_p", (P, 2, TV), F32R), ("bv_p_bc", (P, TV), F32),
    # late weights
    ("b_mret_col", (P, 2), F32),
    ("w3p", (P, 2, P), F32R), ("b3p", (P, 1), F32),
    ("wm3", (P, 2, P), F32R),
    ("wf1", (P, 16, P), F32R), ("bf1", (P, 16), F32),
    ("wf2", (P, 16, P), F32R), ("bf2", (P, 1), F32),
]
A_WEIGHTS = ["wm1", "bm1", "wqm", "bqm", "wkm", "bkm", "wkmp", "bkmp",
             "wvm_c", "bvm_c_bc", "wvm_p", "bvm_p_bc", "b_pret_col", "b3m_col",
             "w1", "b1", "wq", "bq", "wk", "bk", "wv_p", "bv_p_bc"]




def _blob_layout():
    ro, fo = {}, {}
    rc = fc = 0
    for name, shape, dt_ in WEIGHT_SPECS:
        ncol = int(np.prod(shape[1:]))
        if dt_ == F32R:
            ro[name] = (rc, ncol)
            rc += ncol
        else:
            fo[name] = (fc, ncol)
            fc += ncol
    return ro, rc, fo, fc


def _build_program():
    nc = bacc.Bacc("TRN2", target_bir_lowering=False)
    xp_d = nc.dram_tensor("xp", (L, F), F32, kind="ExternalInput")
    xm_d = nc.dram_tensor("xm", (N, F), F32, kind="ExternalInput")
    ro, rc, fo, fc = _blob_layout()
    wd = {
        "wblob_r": nc.dram_tensor("wblob_r", (P, rc), F32R, kind="ExternalInput"),
        "wblob_f": nc.dram_tensor("wblob_f", (P, fc), F32, kind="ExternalInput"),
    }
    ypix_d = nc.dram_tensor("ypix", (L, F), F32, kind="ExternalOutput")
    ymem_d = nc.dram_tensor("ymem", (N, F), F32, kind="ExternalOutput")
    dbg = {}
    if DEBUG:
        for nm, shape in (("d_ret", (P, 2, P)), ("d_dsb", (P, 512)),
                          ("d_mo", (P, P)), ("d_ffn", (P, 16, P)),
                          ("d_av", (2, P, LC)), ("d_m1", (P, 2, P))):
            dbg[nm] = nc.dram_tensor(nm, shape, F32, kind="ExternalOutput")

    with tile.TileContext(nc) as tc:
        _emit(nc, tc, xp_d, xm_d, wd, ypix_d, ymem_d, dbg)
    nc.finalize()
    return nc


def _emit(nc, tc, xp_d, xm_d, wd, ypix_d, ymem_d, dbg=None):
    from contextlib import ExitStack
    ctx = ExitStack()
    with ctx:
        const = ctx.enter_context(tc.tile_pool(name="const", bufs=1))
        persist = ctx.enter_context(tc.tile_pool(name="persist", bufs=1))
        trans = ctx.enter_context(tc.tile_pool(name="trans", bufs=2))
        trans3 = ctx.enter_context(tc.tile_pool(name="trans3", bufs=2))
        deep3 = ctx.enter_context(tc.tile_pool(name="deep3", bufs=4))
        mm_ps = ctx.enter_context(tc.tile_pool(name="mm_ps", bufs=3, space="PSUM"))
        tin_ps = ctx.enter_context(tc.tile_pool(name="tin_ps", bufs=1, space="PSUM"))
        qk_ps = ctx.enter_context(tc.tile_pool(name="qk_ps", bufs=2, space="PSUM"))

        # ---------------- constants / weights in SBUF ----------------
        ro, rc, fo, fc = _blob_layout()
        blob_r = const.tile([P, rc], F32R, tag="blob_r", name="blob_r")
        blob_f = const.tile([P, fc], F32, tag="blob_f", name="blob_f")
        # inputs + first pixel chunks first (the DMA pipe is a FIFO), then
        # weights staged by first use
        xm_nat0 = persist.tile([P, P], F32, tag="xm_nat0", name="xm_nat0")
        nc.sync.dma_start(xm_nat0[:], xm_d[:])
        xp_pre = persist.tile([P, 8, P], F32, tag="xp_pre", name="xp_pre")
        nc.sync.dma_start(xp_pre[:],
                          xp_d[0:1024, :].rearrange("(t p) c -> p t c", p=P))
        actwarm = const.tile([P, 2], F32, tag="actwarm", name="actwarm")
        r0_ = ro["wm1"][0] + ro["wm1"][1]
        f0_ = fo["bm1"][0] + fo["bm1"][1]
        ra = max(off + n_ for off, n_ in (ro[k] for k in A_WEIGHTS if k in ro))
        fa = max(off + n_ for off, n_ in (fo[k] for k in A_WEIGHTS if k in fo))
        nc.sync.dma_start(blob_r[:, :r0_], wd["wblob_r"][:, :r0_])
        nc.sync.dma_start(blob_f[:, :f0_], wd["wblob_f"][:, :f0_])
        nc.sync.dma_start(blob_r[:, r0_:ra], wd["wblob_r"][:, r0_:ra])
        nc.sync.dma_start(blob_f[:, f0_:fa], wd["wblob_f"][:, f0_:fa])
        nc.sync.dma_start(blob_r[:, ra:], wd["wblob_r"][:, ra:])
        nc.sync.dma_start(blob_f[:, fa:], wd["wblob_f"][:, fa:])
        W = {}
        for name, shape, dt_ in WEIGHT_SPECS:
            if dt_ == F32R:
                off, ncol = ro[name]
                ap = blob_r[:, off:off + ncol]
            else:
                off, ncol = fo[name]
                ap = blob_f[:, off:off + ncol]
            if len(shape) == 3:
                ap = ap.rearrange("p (a b) -> p a b", b=shape[2])
            elif len(shape) == 4:
                ap = ap.rearrange("p (a b c) -> p a b c", b=shape[2], c=shape[3])
            W[name] = ap
        ident = const.tile([P, P], F32, tag="ident")
        make_identity(nc, ident[:])
        ones_r = const.tile([P, 2], F32R, tag="ones_r")
        nc.vector.memset(ones_r[:].bitcast(F32), 1.0)
        ones64 = const.tile([P, 64], F32, tag="ones64")
        nc.vector.memset(ones64[:], 1.0)
        nc.scalar.activation(actwarm[:], ones64[:, 0:2], AF.Tanh)

        # ---------------- persistent buffers ----------------
        xpT = persist.tile([P, L], F32R, tag="xpT")            # 16 KB/part
        kcatT = persist.tile([P, L + N], F32R, tag="kcatT")    # 16.5 KB
        vaug = persist.tile([P, MT, 260], F32R, tag="vaug")    # 33.4 KB
        mq_bd = persist.tile([P, H * P], F32R, tag="mq_bd")    # 4 KB
        mvpx_pad = persist.tile([P, 2, 4, P], F32R, tag="mvpx_pad")  # 4 KB
        biaspret = persist.tile([P, 2], F32, tag="biaspret")
        xm_b3T = persist.tile([P, P], F32, tag="xm_b3T")
        mkpT = persist.tile([P, 2, P], F32R, tag="mkpT")
        mvpx = persist.tile([P, TV], F32R, tag="mvpx")
        retT = persist.tile([P, 2, P], F32R, tag="retT")

        # ones columns of vaug (slot 64 of each 65-wide pair block)
        nc.vector.memset(
            vaug[:].rearrange("p t (pr c) -> p t pr c", c=65)[:, :, :, 64:65]
            .bitcast(F32), 1.0)

        # ================= PHASE A: memory front =================
        ps = mm_ps.tile([P, LC], F32, tag="mm")
        nc.tensor.transpose(ps[:, 0:P], xm_nat0[:], ident[:])
        xmT = trans.tile([P, P], F32R, tag="xmT")
        nc.vector.tensor_copy(xmT[:], ps[:, 0:P])
        nc.vector.tensor_scalar(xm_b3T[:], ps[:, 0:P], W["b3m_col"][:], None,
                                ALU.add)

        # M1T feature-major [2][128, 128]
        m1T = persist.tile([P, 2, P], F32R, tag="m1T")
        for mc in range(2):
            pm = mm_ps.tile([P, LC], F32, tag="mm")
            nc.tensor.matmul(pm[:, 0:P], W["wm1"][:, mc, :], xmT[:],
                             start=True, stop=True)
            nc.vector.tensor_scalar(m1T[:, mc, :], pm[:, 0:P],
                                    W["bm1"][:, mc:mc + 1], 0.0, ALU.add, ALU.max)

        # mqT compact -> mq_bd blockdiag
        pm = mm_ps.tile([P, LC], F32, tag="mm")
        for kc in range(2):
            nc.tensor.matmul(pm[:, 0:P], W["wqm"][:, kc, :], m1T[:, kc, :],
                             start=(kc == 0), stop=(kc == 1))
        mqT = trans.tile([P, P], F32R, tag="mqT")
        nc.vector.tensor_scalar(mqT[:], pm[:, 0:P], W["bqm"][:], None, ALU.add)
        nc.vector.memset(mq_bd[:].bitcast(F32), 0.0)
        for h in range(H):
            nc.sync.dma_start(mq_bd[h * DK:(h + 1) * DK, h * P:(h + 1) * P],
                              mqT[h * DK:(h + 1) * DK, :])

        # mkT compact -> kcatT tail
        pm = mm_ps.tile([P, LC], F32, tag="mm")
        for kc in range(2):
            nc.tensor.matmul(pm[:, 0:P], W["wkm"][:, kc, :], m1T[:, kc, :],
                             start=(kc == 0), stop=(kc == 1))
        nc.vector.tensor_scalar(kcatT[:, L:L + N], pm[:, 0:P], W["bkm"][:],
                                None, ALU.add)

        # mk padded (pixel QK lhsT)
        for mc in range(2):
            pm = mm_ps.tile([P, LC], F32, tag="mm")
            for kc in range(2):
                nc.tensor.matmul(pm[:, 0:P], W["wkmp"][:, kc, mc, :],
                                 m1T[:, kc, :], start=(kc == 0), stop=(kc == 1))
            nc.vector.tensor_scalar(mkpT[:, mc, :], pm[:, 0:P],
                                    W["bkmp"][:, mc:mc + 1], None, ALU.add)

        # mv for v_cat -> vaug chunk 32
        pm = mm_ps.tile([P, LC], F32, tag="mm")
        for kc in range(2):
            nc.tensor.matmul(pm[:, 0:TV], m1T[:, kc, :], W["wvm_c"][:, kc, :],
                             start=(kc == 0), stop=(kc == 1))
        nc.vector.tensor_tensor(
            vaug[:, 32].rearrange("p (pr c) -> p pr c", c=65)[:, :, 0:64],
            pm[:, 0:TV].rearrange("p (pr c) -> p pr c", c=64),
            W["bvm_c_bc"][:].rearrange("p (pr c) -> p pr c", c=64), ALU.add)

        # mv for pixel AV (scaled); plus zero-padded per-head variant
        pm = mm_ps.tile([P, LC], F32, tag="mm")
        for kc in range(2):
            nc.tensor.matmul(pm[:, 0:TV], m1T[:, kc, :], W["wvm_p"][:, kc, :],
                             start=(kc == 0), stop=(kc == 1))
        nc.vector.tensor_tensor(mvpx[:], pm[:, 0:TV], W["bvm_p_bc"][:], ALU.add)
        nc.vector.memset(mvpx_pad[:].bitcast(F32), 0.0)
        for h in range(H):
            g_, i_ = divmod(h, 4)
            nc.vector.tensor_copy(
                mvpx_pad[:, g_, i_, 32 * i_:32 * i_ + 32],
                mvpx[:, 32 * h:32 * h + 32])

        # colsum of mvpx (pixel AV bias) + b_pret
        pm = mm_ps.tile([P, LC], F32, tag="mm")
        for c_ in range(2):
            nc.tensor.matmul(pm[:, 2 * c_:2 * c_ + 2],
                             mvpx[:, P * c_:P * (c_ + 1)],
                             ones_r[:], start=True, stop=True)
        nc.vector.tensor_tensor(biaspret[:], pm[:, 0:4:2], W["b_pret_col"][:],
                                ALU.add)

        # ================= PHASE B: pixel pipeline =================
        # Software-pipelined emission: front(c+1) is emitted before tail(c) so
        # the scheduler can fill attention-phase gaps with next-chunk work.
        def _front(c):
            l0 = c * LC
            pst = tin_ps.tile([P, LC], F32, tag="tin", name="pst")
            for j in range(4):
                if c < 2:
                    xnat = xp_pre[:, 4 * c + j, :]
                else:
                    xt_ = trans3.tile([P, P], F32, tag="xnat", name="xnat")
                    nc.sync.dma_start(xt_[:], xp_d[l0 + P * j:l0 + P * (j + 1), :])
                    xnat = xt_[:]
                nc.tensor.transpose(pst[:, P * j:P * (j + 1)], xnat, ident[:])
            nc.vector.tensor_copy(xpT[:, l0:l0 + LC], pst[:])

            # conv1 -> P1T [2][128, 512]
            p1T = deep3.tile([P, 2, LC], F32R, tag="p1T", name="p1T")
            for mc in range(2):
                pm = tin_ps.tile([P, LC], F32, tag="tin", name="pm")
                nc.tensor.matmul(pm[:], W["w1"][:, mc, :],
                                 xpT[:, l0:l0 + LC],
                                 start=True, stop=True)
                nc.scalar.activation(p1T[:, mc, :], pm[:], AF.Relu,
                                     bias=W["b1"][:, mc:mc + 1])

            # qkv projections
            pqTp = deep3.tile([P, 2, LC], F32R, tag="pqTp", name="pqTp")
            for mc in range(2):
                pm = mm_ps.tile([P, LC], F32, tag="mm", name="pm")
                for kc in range(2):
                    nc.tensor.matmul(pm[:], W["wq"][:, kc, mc, :], p1T[:, kc, :],
                                     start=(kc == 0), stop=(kc == 1))
                nc.vector.tensor_scalar(pqTp[:, mc, :], pm[:],
                                        W["bq"][:, mc:mc + 1], None, ALU.add)
            pm = mm_ps.tile([P, LC], F32, tag="mm", name="pm")
            for kc in range(2):
                nc.tensor.matmul(pm[:], W["wk"][:, kc, :], p1T[:, kc, :],
                                 start=(kc == 0), stop=(kc == 1))
            nc.vector.tensor_scalar(kcatT[:, l0:l0 + LC], pm[:], W["bk"][:],
                                    None, ALU.add)
            for lt in range(4):
                pm = mm_ps.tile([P, LC], F32, tag="mm", name="pm")
                for kc in range(2):
                    nc.tensor.matmul(pm[:, 0:TV],
                                     p1T[:, kc, P * lt:P * (lt + 1)],
                                     W["wv_p"][:, kc, :],
                                     start=(kc == 0), stop=(kc == 1))
                nc.vector.tensor_tensor(
                    vaug[:, 4 * c + lt].rearrange("p (pr x) -> p pr x", x=65)
                    [:, :, 0:64],
                    pm[:, 0:TV].rearrange("p (pr x) -> p pr x", x=64),
                    W["bv_p_bc"][:].rearrange("p (pr x) -> p pr x", x=64),
                    ALU.add)
            return pqTp

        def _tail(c, pqTp):
            l0 = c * LC
            # pixel QK (row-packed strips, 2 heads per round) + tanh,
            # then AV per 4-head group (4 accumulating zero-padded MMs)
            pretT = trans.tile([P, 2, LC], F32R, tag="pretT", name="pretT")
            for g_ in range(2):
                pattn = trans.tile([P, 4, LC], F32R, tag="pattn", name="pattn")
                for r in range(2):
                    pq_ = qk_ps.tile([P, 2 * LC], F32, tag="qk", name="pq_")
                    for i in range(2):
                        h = 4 * g_ + 2 * r + i
                        pos = 32 * (h % 4)
                        nc.tensor.matmul(pq_[:, LC * i:LC * (i + 1)],
                                         mkpT[pos:pos + 32, h // 4, :],
                                         pqTp[pos:pos + 32, h // 4, :],
                                         start=True, stop=True,
                                         tile_position=(pos, 0))
                    nc.scalar.activation(
                        pattn[:, 2 * r:2 * r + 2, :]
                        .rearrange("p a b -> p (a b)"),
                        pq_[:], AF.Tanh, scale=0.5)
                pm = mm_ps.tile([P, LC], F32, tag="mm", name="pm")
                for i in range(4):
                    nc.tensor.matmul(pm[:], mvpx_pad[:, g_, i, :],
                                     pattn[:, i, :],
                                     start=(i == 0), stop=(i == 3))
                nc.vector.tensor_scalar(pretT[:, g_, :], pm[:],
                                        biaspret[:, g_:g_ + 1], 0.0,
                                        ALU.add, ALU.max)

            # conv3 + residual + relu (feature-major), then transpose out
            pm = mm_ps.tile([P, LC], F32, tag="mm", name="pm")
            for kc in range(2):
                nc.tensor.matmul(pm[:], W["w3p"][:, kc, :], pretT[:, kc, :],
                                 start=(kc == 0), stop=(kc == 1))
            poutT = trans.tile([P, LC], F32, tag="poutT", name="poutT")
            nc.vector.tensor_tensor(poutT[:], pm[:], xpT[:, l0:l0 + LC], ALU.add)
            nc.vector.tensor_scalar(poutT[:], poutT[:], W["b3p"][:], 0.0,
                                    ALU.add, ALU.max)
            pst2 = mm_ps.tile([P, LC], F32, tag="mm", name="pst2")
            for j in range(4):
                nc.tensor.transpose(pst2[:, P * j:P * (j + 1)],
                                    poutT[:, P * j:P * (j + 1)], ident[:])
            pout = trans.tile([P, LC], F32, tag="poutT", name="pout")
            nc.vector.tensor_copy(pout[:], pst2[:])
            for j in range(4):
                nc.sync.dma_start(ypix_d[l0 + P * j:l0 + P * (j + 1), :],
                                  pout[:, P * j:P * (j + 1)])

        pend = {}
        for c in range(NCHUNK + 1):
            if c < NCHUNK:
                pend[c] = _front(c)
            if c >= 1:
                _tail(c - 1, pend.pop(c - 1))

        # ================= PHASE C: memory attention =================
        avp = [mm_ps.tile([P, LC], F32, tag="mm", name=f"av{q}") for q in range(2)]
        for t in range(MT):
            pqm = qk_ps.tile([P, 2 * LC], F32, tag="qk")
            for u in range(2):
                nc.tensor.matmul(pqm[:, LC * u:LC * (u + 1)],
                                 kcatT[:, P * t:P * (t + 1)],
                                 mq_bd[:, LC * u:LC * (u + 1)],
                                 start=True, stop=True)
            probs = trans3.tile([P, 2 * LC], F32R, tag="probs")
            nc.scalar.activation(probs[:], pqm[:], AF.Exp)
            for pr in range(4):
                nc.tensor.matmul(
                    avp[pr // 2][0:65, 256 * (pr % 2):256 * (pr % 2) + 256],
                    vaug[:, t, 65 * pr:65 * pr + 65],
                    probs[:, 256 * pr:256 * (pr + 1)],
                    start=(t == 0), stop=(t == MT - 1))

        # ================= PHASE D: memory tail =================
        # denominator rows: one pair at a time through a small row-64 buffer
        dsb = persist.tile([P, 512], F32, tag="dsb")
        for pr in range(4):
            q, s_ = divmod(pr, 2)
            col = 256 * (pr % 2)
            nc.vector.tensor_copy(dsb[64:65, col:col + 256],
                                  avp[q][64:65, 256 * s_:256 * (s_ + 1)])
            psd = qk_ps.tile([P, 2 * LC], F32, tag="qk")
            nc.tensor.matmul(psd[0:64, 0:256], ones64[64:65, :],
                             dsb[64:65, col:col + 256],
                             start=True, stop=True, tile_position=(64, 0))
            recip = trans3.tile([64, 256], F32, tag="recip")
            nc.vector.reciprocal(recip[:], psd[0:64, 0:256])
            ro = 64 * s_
            nc.vector.tensor_tensor(retT[ro:ro + 32, q, :],
                                    avp[q][0:32, 256 * s_:256 * s_ + P],
                                    recip[0:32, 0:P], ALU.mult)
            nc.vector.tensor_tensor(retT[ro + 32:ro + 64, q, :],
                                    avp[q][32:64, 256 * s_ + P:256 * (s_ + 1)],
                                    recip[32:64, P:2 * P], ALU.mult)
        for q in range(2):
            nc.vector.tensor_scalar(retT[:, q, :], retT[:, q, :],
                                    W["b_mret_col"][:, q:q + 1], 0.0,
                                    ALU.add, ALU.max)

        if dbg:
            for q in range(2):
                dd = trans.tile([P, LC], F32, tag="poutT", name="dd")[:, 0:P]
                nc.vector.tensor_copy(dd[:], retT[:, q, :])
                nc.sync.dma_start(dbg["d_ret"][:, q, :], dd[:])
                da = trans.tile([P, LC], F32, tag="poutT", name="da")
                nc.vector.tensor_copy(da[:], avp[q][:])
                nc.sync.dma_start(dbg["d_av"][q], da[:])
                dm1 = trans.tile([P, LC], F32, tag="poutT", name="dm1")[:, 0:P]
                nc.vector.tensor_copy(dm1[:], m1T[:, q, :])
                nc.sync.dma_start(dbg["d_m1"][:, q, :], dm1[:])
            nc.sync.dma_start(dbg["d_dsb"][:, 0:256], dsb[:, 0:256])

        # mem conv3 feature-major (+ residual + relu) -> moT directly
        pm = mm_ps.tile([P, LC], F32, tag="mm")
        for kc in range(2):
            nc.tensor.matmul(pm[:, 0:P], W["wm3"][:, kc, :], retT[:, kc, :],
                             start=(kc == 0), stop=(kc == 1))
        moT = trans.tile([P, P], F32, tag="moT")
        nc.vector.tensor_tensor(moT[:], pm[:, 0:P], xm_b3T[:], ALU.add)
        nc.vector.tensor_scalar(moT[:], moT[:], 0.0, None, ALU.max)
        moTr = trans.tile([P, P], F32R, tag="moTr")
        nc.vector.tensor_copy(moTr[:], moT[:])

        if dbg:
            dmo = trans.tile([P, LC], F32, tag="poutT", name="dmo")[:, 0:P]
            nc.vector.tensor_copy(dmo[:], mo[:])
            nc.sync.dma_start(dbg["d_mo"][:], dmo[:])

        # ffn1 -> ffnT [128, 16, 128]
        ffnT = persist.tile([P, 16, P], F32R, tag="ffnT")
        for g_ in range(4):
            pm = mm_ps.tile([P, LC], F32, tag="mm")
            for s_ in range(4):
                mc = 4 * g_ + s_
                nc.tensor.matmul(pm[:, P * s_:P * (s_ + 1)],
                                 W["wf1"][:, mc, :], moTr[:],
                                 start=True, stop=True)
            for s_ in range(4):
                mc = 4 * g_ + s_
                nc.scalar.activation(ffnT[:, mc, :],
                                     pm[:, P * s_:P * (s_ + 1)], AF.Relu,
                                     bias=W["bf1"][:, mc:mc + 1])
        if dbg:
            for mc in range(16):
                df = trans.tile([P, LC], F32, tag="poutT", name="df")[:, 0:P]
                nc.vector.tensor_copy(df[:], ffnT[:, mc, :])
                nc.sync.dma_start(dbg["d_ffn"][:, mc, :], df[:])

        # ffn2 (+ residual + relu) -> transpose -> ymem
        # two independent accumulators so the chain overlaps ffn1 production
        pma = mm_ps.tile([P, LC], F32, tag="mm", name="pma")
        pmb = mm_ps.tile([P, LC], F32, tag="mm", name="pmb")
        for kc in range(8):
            nc.tensor.matmul(pma[:, 0:P], W["wf2"][:, kc, :], ffnT[:, kc, :],
                             start=(kc == 0), stop=(kc == 7))
        for kc in range(8, 16):
            nc.tensor.matmul(pmb[:, 0:P], W["wf2"][:, kc, :], ffnT[:, kc, :],
                             start=(kc == 8), stop=(kc == 15))
        mo2T = trans.tile([P, P], F32, tag="mo2T")
        nc.vector.tensor_tensor(mo2T[:], pma[:, 0:P], moT[:], ALU.add)
        nc.vector.tensor_tensor(mo2T[:], pmb[:, 0:P], mo2T[:], ALU.add)
        nc.vector.tensor_scalar(mo2T[:], mo2T[:], W["bf2"][:], 0.0,
                                ALU.add, ALU.max)
        pst = mm_ps.tile([P, LC], F32, tag="mm")
        nc.tensor.transpose(pst[:, 0:P], mo2T[:], ident[:])
        mo2 = trans.tile([P, P], F32, tag="mo2")
        nc.vector.tensor_copy(mo2[:], pst[:, 0:P])
        nc.sync.dma_start(ymem_d[:], mo2[:])


def kernel(**inputs):
    if "nc" not in _cached:
        _cached["nc"] = _build_program()
    nc = _cached["nc"]
    d = _prep_host(inputs)
    pix = np.ascontiguousarray(np.asarray(inputs["pixel_input"], np.float32))
    mem = np.ascontiguousarray(np.asarray(inputs["memory_input"], np.float32))
    in_maps = []
    for b in range(B):
        m = {"xp": pix[b], "xm": mem[b]}
        m.update(d)
        in_maps.append(m)
    res = run_bass_kernel_spmd(nc, in_maps, core_ids=list(range(B)))
    pix_out = np.stack([res.results[b]["ypix"] for b in range(B)])
    mem_out = np.stack([res.results[b]["ymem"] for b in range(B)])
    return pix_out, mem_out


# revision 46
# speedup vs baseline: 1.0798x; 1.0638x over previous
"""Trainium2 Bass kernel for nn_DualPathTransformerLayer.

Sharding: data-parallel over batch -- B=8 batch elements, one per NeuronCore.
Each core runs an identical single-core program (SPMD) on its own batch slice;
weights are broadcast. No collectives needed.

Single-core program layout summary:
  - activations kept feature-major ("T" = [channels, positions]) where matmuls
    consume them, natural where needed (values for attention, outputs)
  - all BN affines folded into weights/bias host-side (inference BN)
  - pixel attention: sigmoid(x) = 0.5 + 0.5*tanh(x/2); the 0.5 factors and the
    BN-ret gamma are folded into the value projection; tanh on ACT engine
  - memory attention: softmax without max-subtraction (|logits| <= ~9);
    logits computed transposed [keys, (head, query)] via a block-diagonal
    query operand; denominator rides the AV matmul as an extra ones column
  - matmuls in float32r (1 cyc/row at N>=256) except tiny ones
"""
import numpy as np
import concourse.bass as bass
from concourse import bacc
import concourse.mybir as mybir
import concourse.tile as tile
from concourse.bass_utils import run_bass_kernel_spmd
from concourse.masks import make_identity

# problem dims (hardcoded per contract)
B, L, N, F = 8, 4096, 128, 128
H, DK, DV = 8, 16, 32
TK, TV = H * DK, H * DV          # 128, 256
BOT, FFN = 256, 2048
EPS = 1e-3
INV = float(1.0 / np.sqrt(1.0 + EPS))
P = 128
NCHUNK = 8          # pixel l-chunks of 512
LC = 512            # l-chunk size
MT = (L + N) // P   # 33 m-tiles for memory attention

F32 = mybir.dt.float32
F32R = mybir.dt.float32r
AF = mybir.ActivationFunctionType
ALU = mybir.AluOpType

_cached = {}
DEBUG = False


def _prep_host(w):
    """Fold BN affines into weights/biases. Returns dict of np arrays."""
    d = {}
    g = lambda p: np.asarray(p[0], np.float32) * INV
    be = lambda p: np.asarray(p[1], np.float32)

    def f32(x):
        return np.ascontiguousarray(x, np.float32)

    # ---- pixel conv1 ----
    g1, b1 = g(w["bn_pix1"]), be(w["bn_pix1"])
    W1 = np.asarray(w["W_pix1"], np.float32) * g1[None, :]          # [128,256]
    d["w1"] = f32(W1.reshape(F, 2, P))                              # lhsT [c, mc, m]
    d["b1"] = f32(b1.reshape(2, P).T)                               # [128, 2]

    # ---- pixel qkv ----
    gq, bq = g(w["bn_pix_qkv"]), be(w["bn_pix_qkv"])
    Wq = np.asarray(w["W_pix_qkv"], np.float32)
    Wqk = Wq * gq[None, :]
    bqk = bq.copy()
    # fold pixel-sim gamma into pixel q columns (per head)
    gs_pix = g(w["bn_pix_sim"])        # [H]
    bs_pix = be(w["bn_pix_sim"])       # [H]
    Wq_q = Wqk[:, :TK].copy()
    bq_q = bqk[:TK].copy()
    for h in range(H):
        Wq_q[:, h * DK:(h + 1) * DK] *= gs_pix[h]
        bq_q[h * DK:(h + 1) * DK] *= gs_pix[h]
    # pad q to 32-per-head; slot 16 is the constant-1 channel (bias-in-matmul)
    Wq_pad = np.zeros((BOT, H * 32), np.float32)
    bq_pad = np.zeros(H * 32, np.float32)
    for h in range(H):
        Wq_pad[:, h * 32:h * 32 + DK] = Wq_q[:, h * DK:(h + 1) * DK]
        bq_pad[h * 32:h * 32 + DK] = bq_q[h * DK:(h + 1) * DK]
        bq_pad[h * 32 + DK] = 1.0
    d["wq"] = f32(Wq_pad.reshape(2, P, 2, P).transpose(1, 0, 2, 3))  # [c,kc,mc,m]
    d["bq"] = f32(bq_pad.reshape(2, P).T)                            # [128,2]
    # k compact
    d["wk"] = f32(Wqk[:, TK:2 * TK].reshape(2, P, P).transpose(1, 0, 2))  # [c,kc,m]
    d["bk"] = f32(bqk[TK:2 * TK].reshape(P, 1))
    # v: fold mem-ret gamma (pv feeds only the memory-path v_cat)
    g_mret = g(w["bn_mem_ret"]).reshape(TV)      # [(h,dv)]
    b_mret = be(w["bn_mem_ret"]).reshape(TV)
    Wv_p = Wqk[:, 2 * TK:] * g_mret[None, :]
    bv_p = bqk[2 * TK:] * g_mret
    d["wv_p"] = f32(Wv_p.reshape(2, P, TV).transpose(1, 0, 2))       # [c,kc,n]
    d["bv_p_bc"] = f32(np.tile(bv_p[None, :], (P, 1)))               # [128,256]
    d["b_mret_col"] = f32(b_mret.reshape(2, P).T)                    # [128,2]

    # ---- memory conv1 ----
    gm1, bm1 = g(w["bn_mem1"]), be(w["bn_mem1"])
    Wm1 = np.asarray(w["W_mem1"], np.float32) * gm1[None, :]
    d["wm1"] = f32(Wm1.reshape(F, 2, P))
    d["bm1"] = f32(bm1.reshape(2, P).T)

    # ---- memory qkv ----
    gqm, bqm = g(w["bn_mem_qkv"]), be(w["bn_mem_qkv"])
    Wqm = np.asarray(w["W_mem_qkv"], np.float32) * gqm[None, :]
    bqm_f = bqm.copy()
    gs_mem = g(w["bn_mem_sim"])
    # (mem-sim beta cancels in softmax -- dropped)
    Wq_m = Wqm[:, :TK].copy()
    bq_m = bqm_f[:TK].copy()
    for h in range(H):
        Wq_m[:, h * DK:(h + 1) * DK] *= gs_mem[h]
        bq_m[h * DK:(h + 1) * DK] *= gs_mem[h]
    d["wqm"] = f32(Wq_m.reshape(2, P, P).transpose(1, 0, 2))
    d["bqm"] = f32(bq_m.reshape(P, 1))
    # mk compact (for mem-path k_cat)
    d["wkm"] = f32(Wqm[:, TK:2 * TK].reshape(2, P, P).transpose(1, 0, 2))
    d["bkm"] = f32(bqm_f[TK:2 * TK].reshape(P, 1))
    # mk padded (for pixel QK strips; slot 16 carries pixel-sim beta)
    Wk_m = Wqm[:, TK:2 * TK]
    Wk_pad = np.zeros((BOT, H * 32), np.float32)
    bk_pad = np.zeros(H * 32, np.float32)
    for h in range(H):
        Wk_pad[:, h * 32:h * 32 + DK] = Wk_m[:, h * DK:(h + 1) * DK]
        bk_pad[h * 32:h * 32 + DK] = bqm_f[TK + h * DK:TK + (h + 1) * DK]
        bk_pad[h * 32 + DK] = bs_pix[h]
    d["wkmp"] = f32(Wk_pad.reshape(2, P, 2, P).transpose(1, 0, 2, 3))
    d["bkmp"] = f32(bk_pad.reshape(2, P).T)
    # mv for v_cat (mem-ret gamma folded)
    Wv_m = Wqm[:, 2 * TK:]
    bv_m = bqm_f[2 * TK:]
    Wv_mc = Wv_m * g_mret[None, :]
    bv_mc = bv_m * g_mret
    d["wvm_c"] = f32(Wv_mc.reshape(2, P, TV).transpose(1, 0, 2))
    d["bvm_c_bc"] = f32(np.tile(bv_mc[None, :], (P, 1)))
    # mv for pixel AV (0.5 * pix-ret gamma folded)
    g_pret = g(w["bn_pix_ret"]).reshape(TV)
    b_pret = be(w["bn_pix_ret"]).reshape(TV)
    Wv_mp = Wv_m * (0.5 * g_pret)[None, :]
    bv_mp = bv_m * (0.5 * g_pret)
    d["wvm_p"] = f32(Wv_mp.reshape(2, P, TV).transpose(1, 0, 2))
    d["bvm_p_bc"] = f32(np.tile(bv_mp[None, :], (P, 1)))
    d["b_pret_col"] = f32(b_pret.reshape(2, P).T)                    # [128,2]

    # ---- conv3 ----
    g3, b3 = g(w["bn_pix3"]), be(w["bn_pix3"])
    W3 = np.asarray(w["W_pix3"], np.float32) * g3[None, :]           # [256,128]
    d["w3p"] = f32(W3.reshape(2, P, P).transpose(1, 0, 2))           # [c,kc,m]
    d["b3p"] = f32(b3.reshape(P, 1))
    g3m, b3m = g(w["bn_mem3"]), be(w["bn_mem3"])
    W3m = np.asarray(w["W_mem3"], np.float32) * g3m[None, :]
    d["wm3"] = f32(W3m.reshape(2, P, P).transpose(1, 0, 2))
    d["b3m_col"] = f32(b3m.reshape(P, 1))

    # ---- ffn ----
    gf1, bf1 = g(w["bn_ffn1"]), be(w["bn_ffn1"])
    Wf1 = np.asarray(w["W_ffn1"], np.float32) * gf1[None, :]         # [128,2048]
    d["wf1"] = f32(Wf1.reshape(F, 16, P))                            # lhsT [c,mc,m]
    d["bf1"] = f32(bf1.reshape(16, P).T)                             # [128,16]
    gf2, bf2 = g(w["bn_ffn2"]), be(w["bn_ffn2"])
    Wf2 = np.asarray(w["W_ffn2"], np.float32) * gf2[None, :]         # [2048,128]
    d["wf2"] = f32(Wf2.reshape(16, P, P).transpose(1, 0, 2))         # [c,kc,m]
    d["bf2"] = f32(bf2.reshape(P, 1))

    # concat everything into two blobs ([128, X] each) for 2 big DMAs
    rcols, fcols = [], []
    offs = {}
    for name, shape, dt_ in WEIGHT_SPECS:
        a = d[name].reshape(P, -1)
        tgt = rcols if dt_ == F32R else fcols
        off = sum(x.shape[1] for x in tgt)
        offs[name] = off
        tgt.append(a)
    out = {"wblob_r": np.ascontiguousarray(np.concatenate(rcols, axis=1)),
           "wblob_f": np.ascontiguousarray(np.concatenate(fcols, axis=1))}
    return out


WEIGHT_SPECS = [
    # phase A (memory front) first -- their DMA is staged ahead
    ("wm1", (P, 2, P), F32R), ("bm1", (P, 2), F32),
    ("wqm", (P, 2, P), F32R), ("bqm", (P, 1), F32),
    ("wkm", (P, 2, P), F32R), ("bkm", (P, 1), F32),
    ("wkmp", (P, 2, 2, P), F32R), ("bkmp", (P, 2), F32),
    ("wvm_c", (P, 2, TV), F32R), ("bvm_c_bc", (P, TV), F32),
    ("wvm_p", (P, 2, TV), F32R), ("bvm_p_bc", (P, TV), F32),
    ("b_pret_col", (P, 2), F32), ("b3m_col", (P, 1), F32),
    ("w1", (P, 2, P), F32R), ("b1", (P, 2), F32),
    ("wq", (P, 2, 2, P), F32R), ("bq", (P, 2), F32),
    ("wk", (P, 2, P), F32R), ("bk", (P, 1), F32),
    ("wv_p", (P, 2, TV), F32R), ("bv_p_bc", (P, TV), F32),
    # late weights
    ("b_mret_col", (P, 2), F32),
    ("w3p", (P, 2, P), F32R), ("b3p", (P, 1), F32),
    ("wm3", (P, 2, P), F32R),
    ("wf1", (P, 16, P), F32R), ("bf1", (P, 16), F32),
    ("wf2", (P, 16, P), F32R), ("bf2", (P, 1), F32),
]
A_WEIGHTS = ["wm1", "bm1", "wqm", "bqm", "wkm", "bkm", "wkmp", "bkmp",
             "wvm_c", "bvm_c_bc", "wvm_p", "bvm_p_bc", "b_pret_col", "b3m_col",
             "w1", "b1", "wq", "bq", "wk", "bk", "wv_p", "bv_p_bc"]




def _blob_layout():
    ro, fo = {}, {}
    rc = fc = 0
    for name, shape, dt_ in WEIGHT_SPECS:
        ncol = int(np.prod(shape[1:]))
        if dt_ == F32R:
            ro[name] = (rc, ncol)
            rc += ncol
        else:
            fo[name] = (fc, ncol)
            fc += ncol
    return ro, rc, fo, fc


def _build_program():
    nc = bacc.Bacc("TRN2", target_bir_lowering=False)
    xp_d = nc.dram_tensor("xp", (L, F), F32, kind="ExternalInput")
    xm_d = nc.dram_tensor("xm", (N, F), F32, kind="ExternalInput")
    ro, rc, fo, fc = _blob_layout()
    wd = {
        "wblob_r": nc.dram_tensor("wblob_r", (P, rc), F32R, kind="ExternalInput"),
        "wblob_f": nc.dram_tensor("wblob_f", (P, fc), F32, kind="ExternalInput"),
    }
    ypix_d = nc.dram_tensor("ypix", (L, F), F32, kind="ExternalOutput")
    ymem_d = nc.dram_tensor("ymem", (N, F), F32, kind="ExternalOutput")
    dbg = {}
    if DEBUG:
        for nm, shape in (("d_ret", (P, 2, P)), ("d_dsb", (P, 512)),
                          ("d_mo", (P, P)), ("d_ffn", (P, 16, P)),
                          ("d_av", (2, P, LC)), ("d_m1", (P, 2, P))):
            dbg[nm] = nc.dram_tensor(nm, shape, F32, kind="ExternalOutput")

    with tile.TileContext(nc) as tc:
        _emit(nc, tc, xp_d, xm_d, wd, ypix_d, ymem_d, dbg)
    nc.finalize()
    return nc


def _emit(nc, tc, xp_d, xm_d, wd, ypix_d, ymem_d, dbg=None):
    from contextlib import ExitStack
    ctx = ExitStack()
    with ctx:
        const = ctx.enter_context(tc.tile_pool(name="const", bufs=1))
        persist = ctx.enter_context(tc.tile_pool(name="persist", bufs=1))
        trans = ctx.enter_context(tc.tile_pool(name="trans", bufs=2))
        trans3 = ctx.enter_context(tc.tile_pool(name="trans3", bufs=2))
        deep3 = ctx.enter_context(tc.tile_pool(name="deep3", bufs=4))
        mm_ps = ctx.enter_context(tc.tile_pool(name="mm_ps", bufs=3, space="PSUM"))
        tin_ps = ctx.enter_context(tc.tile_pool(name="tin_ps", bufs=1, space="PSUM"))
        qk_ps = ctx.enter_context(tc.tile_pool(name="qk_ps", bufs=2, space="PSUM"))

        # ---------------- constants / weights in SBUF ----------------
        ro, rc, fo, fc = _blob_layout()
        blob_r = const.tile([P, rc], F32R, tag="blob_r", name="blob_r")
        blob_f = const.tile([P, fc], F32, tag="blob_f", name="blob_f")
        # inputs + first pixel chunks first (the DMA pipe is a FIFO), then
        # weights staged by first use
        xm_nat0 = persist.tile([P, P], F32, tag="xm_nat0", name="xm_nat0")
        nc.sync.dma_start(xm_nat0[:], xm_d[:])
        xp_pre = persist.tile([P, 8, P], F32, tag="xp_pre", name="xp_pre")
        nc.sync.dma_start(xp_pre[:],
                          xp_d[0:1024, :].rearrange("(t p) c -> p t c", p=P))
        actwarm = const.tile([P, 2], F32, tag="actwarm", name="actwarm")
        r0_ = ro["wm1"][0] + ro["wm1"][1]
        f0_ = fo["bm1"][0] + fo["bm1"][1]
        ra = max(off + n_ for off, n_ in (ro[k] for k in A_WEIGHTS if k in ro))
        fa = max(off + n_ for off, n_ in (fo[k] for k in A_WEIGHTS if k in fo))
        nc.sync.dma_start(blob_r[:, :r0_], wd["wblob_r"][:, :r0_])
        nc.sync.dma_start(blob_f[:, :f0_], wd["wblob_f"][:, :f0_])
        nc.sync.dma_start(blob_r[:, r0_:ra], wd["wblob_r"][:, r0_:ra])
        nc.sync.dma_start(blob_f[:, f0_:fa], wd["wblob_f"][:, f0_:fa])
        nc.sync.dma_start(blob_r[:, ra:], wd["wblob_r"][:, ra:])
        nc.sync.dma_start(blob_f[:, fa:], wd["wblob_f"][:, fa:])
        W = {}
        for name, shape, dt_ in WEIGHT_SPECS:
            if dt_ == F32R:
                off, ncol = ro[name]
                ap = blob_r[:, off:off + ncol]
            else:
                off, ncol = fo[name]
                ap = blob_f[:, off:off + ncol]
            if len(shape) == 3:
                ap = ap.rearrange("p (a b) -> p a b", b=shape[2])
            elif len(shape) == 4:
                ap = ap.rearrange("p (a b c) -> p a b c", b=shape[2], c=shape[3])
            W[name] = ap
        ident = const.tile([P, P], F32, tag="ident")
        make_identity(nc, ident[:])
        ones_r = const.tile([P, 2], F32R, tag="ones_r")
        nc.vector.memset(ones_r[:].bitcast(F32), 1.0)
        ones64 = const.tile([P, 64], F32, tag="ones64")
        nc.vector.memset(ones64[:], 1.0)
        nc.scalar.activation(actwarm[:], ones64[:, 0:2], AF.Tanh)

        # ---------------- persistent buffers ----------------
        xpT = persist.tile([P, L], F32R, tag="xpT")            # 16 KB/part
        kcatT = persist.tile([P, L + N], F32R, tag="kcatT")    # 16.5 KB
        vaug = persist.tile([P, MT, 260], F32R, tag="vaug")    # 33.4 KB
        mq_bd = persist.tile([P, H * P], F32R, tag="mq_bd")    # 4 KB
        mvpx_pad = persist.tile([P, 2, 4, P], F32R, tag="mvpx_pad")  # 4 KB
        biaspret = persist.tile([P, 2], F32, tag="biaspret")
        xm_b3T = persist.tile([P, P], F32, tag="xm_b3T")
        mkpT = persist.tile([P, 2, P], F32R, tag="mkpT")
        mvpx = persist.tile([P, TV], F32R, tag="mvpx")
        retT = persist.tile([P, 2, P], F32R, tag="retT")

        # ones columns of vaug (slot 64 of each 65-wide pair block)
        nc.vector.memset(
            vaug[:].rearrange("p t (pr c) -> p t pr c", c=65)[:, :, :, 64:65]
            .bitcast(F32), 1.0)

        # ================= PHASE A: memory front =================
        ps = mm_ps.tile([P, LC], F32, tag="mm")
        nc.tensor.transpose(ps[:, 0:P], xm_nat0[:], ident[:])
        xmT = trans.tile([P, P], F32R, tag="xmT")
        nc.vector.tensor_copy(xmT[:], ps[:, 0:P])
        nc.vector.tensor_scalar(xm_b3T[:], ps[:, 0:P], W["b3m_col"][:], None,
                                ALU.add)

        # M1T feature-major [2][128, 128]
        m1T = persist.tile([P, 2, P], F32R, tag="m1T")
        for mc in range(2):
            pm = mm_ps.tile([P, LC], F32, tag="mm")
            nc.tensor.matmul(pm[:, 0:P], W["wm1"][:, mc, :], xmT[:],
                             start=True, stop=True)
            nc.vector.tensor_scalar(m1T[:, mc, :], pm[:, 0:P],
                                    W["bm1"][:, mc:mc + 1], 0.0, ALU.add, ALU.max)

        # mqT compact -> mq_bd blockdiag
        pm = mm_ps.tile([P, LC], F32, tag="mm")
        for kc in range(2):
            nc.tensor.matmul(pm[:, 0:P], W["wqm"][:, kc, :], m1T[:, kc, :],
                             start=(kc == 0), stop=(kc == 1))
        mqT = trans.tile([P, P], F32R, tag="mqT")
        nc.vector.tensor_scalar(mqT[:], pm[:, 0:P], W["bqm"][:], None, ALU.add)
        nc.vector.memset(mq_bd[:].bitcast(F32), 0.0)
        for h in range(H):
            nc.sync.dma_start(mq_bd[h * DK:(h + 1) * DK, h * P:(h + 1) * P],
                              mqT[h * DK:(h + 1) * DK, :])

        # mkT compact -> kcatT tail
        pm = mm_ps.tile([P, LC], F32, tag="mm")
        for kc in range(2):
            nc.tensor.matmul(pm[:, 0:P], W["wkm"][:, kc, :], m1T[:, kc, :],
                             start=(kc == 0), stop=(kc == 1))
        nc.vector.tensor_scalar(kcatT[:, L:L + N], pm[:, 0:P], W["bkm"][:],
                                None, ALU.add)

        # mk padded (pixel QK lhsT)
        for mc in range(2):
            pm = mm_ps.tile([P, LC], F32, tag="mm")
            for kc in range(2):
                nc.tensor.matmul(pm[:, 0:P], W["wkmp"][:, kc, mc, :],
                                 m1T[:, kc, :], start=(kc == 0), stop=(kc == 1))
            nc.vector.tensor_scalar(mkpT[:, mc, :], pm[:, 0:P],
                                    W["bkmp"][:, mc:mc + 1], None, ALU.add)

        # mv for v_cat -> vaug chunk 32
        pm = mm_ps.tile([P, LC], F32, tag="mm")
        for kc in range(2):
            nc.tensor.matmul(pm[:, 0:TV], m1T[:, kc, :], W["wvm_c"][:, kc, :],
                             start=(kc == 0), stop=(kc == 1))
        nc.vector.tensor_tensor(
            vaug[:, 32].rearrange("p (pr c) -> p pr c", c=65)[:, :, 0:64],
            pm[:, 0:TV].rearrange("p (pr c) -> p pr c", c=64),
            W["bvm_c_bc"][:].rearrange("p (pr c) -> p pr c", c=64), ALU.add)

        # mv for pixel AV (scaled); plus zero-padded per-head variant
        pm = mm_ps.tile([P, LC], F32, tag="mm")
        for kc in range(2):
            nc.tensor.matmul(pm[:, 0:TV], m1T[:, kc, :], W["wvm_p"][:, kc, :],
                             start=(kc == 0), stop=(kc == 1))
        nc.vector.tensor_tensor(mvpx[:], pm[:, 0:TV], W["bvm_p_bc"][:], ALU.add)
        nc.vector.memset(mvpx_pad[:].bitcast(F32), 0.0)
        for h in range(H):
            g_, i_ = divmod(h, 4)
            nc.vector.tensor_copy(
                mvpx_pad[:, g_, i_, 32 * i_:32 * i_ + 32],
                mvpx[:, 32 * h:32 * h + 32])

        # colsum of mvpx (pixel AV bias) + b_pret
        pm = mm_ps.tile([P, LC], F32, tag="mm")
        for c_ in range(2):
            nc.tensor.matmul(pm[:, 2 * c_:2 * c_ + 2],
                             mvpx[:, P * c_:P * (c_ + 1)],
                             ones_r[:], start=True, stop=True)
        nc.vector.tensor_tensor(biaspret[:], pm[:, 0:4:2], W["b_pret_col"][:],
                                ALU.add)

        # ================= PHASE B: pixel pipeline =================
        # Software-pipelined emission: front(c+1) is emitted before tail(c) so
        # the scheduler can fill attention-phase gaps with next-chunk work.
        def _front(c):
            l0 = c * LC
            pst = tin_ps.tile([P, LC], F32, tag="tin", name="pst")
            for j in range(4):
                if c < 2:
                    xnat = xp_pre[:, 4 * c + j, :]
                else:
                    xt_ = trans3.tile([P, P], F32, tag="xnat", name="xnat")
                    nc.sync.dma_start(xt_[:], xp_d[l0 + P * j:l0 + P * (j + 1), :])
                    xnat = xt_[:]
                nc.tensor.transpose(pst[:, P * j:P * (j + 1)], xnat, ident[:])
            nc.vector.tensor_copy(xpT[:, l0:l0 + LC], pst[:])

            # conv1 -> P1T [2][128, 512]
            p1T = deep3.tile([P, 2, LC], F32R, tag="p1T", name="p1T")
            for mc in range(2):
                pm = tin_ps.tile([P, LC], F32, tag="tin", name="pm")
                nc.tensor.matmul(pm[:], W["w1"][:, mc, :],
                                 xpT[:, l0:l0 + LC],
                                 start=True, stop=True)
                nc.scalar.activation(p1T[:, mc, :], pm[:], AF.Relu,
                                     bias=W["b1"][:, mc:mc + 1])

            # qkv projections
            pqTp = deep3.tile([P, 2, LC], F32R, tag="pqTp", name="pqTp")
            for mc in range(2):
                pm = mm_ps.tile([P, LC], F32, tag="mm", name="pm")
                for kc in range(2):
                    nc.tensor.matmul(pm[:], W["wq"][:, kc, mc, :], p1T[:, kc, :],
                                     start=(kc == 0), stop=(kc == 1))
                nc.vector.tensor_scalar(pqTp[:, mc, :], pm[:],
                                        W["bq"][:, mc:mc + 1], None, ALU.add)
            pm = mm_ps.tile([P, LC], F32, tag="mm", name="pm")
            for kc in range(2):
                nc.tensor.matmul(pm[:], W["wk"][:, kc, :], p1T[:, kc, :],
                                 start=(kc == 0), stop=(kc == 1))
            nc.vector.tensor_scalar(kcatT[:, l0:l0 + LC], pm[:], W["bk"][:],
                                    None, ALU.add)
            for lt in range(4):
                pm = mm_ps.tile([P, LC], F32, tag="mm", name="pm")
                for kc in range(2):
                    nc.tensor.matmul(pm[:, 0:TV],
                                     p1T[:, kc, P * lt:P * (lt + 1)],
                                     W["wv_p"][:, kc, :],
                                     start=(kc == 0), stop=(kc == 1))
                nc.vector.tensor_tensor(
                    vaug[:, 4 * c + lt].rearrange("p (pr x) -> p pr x", x=65)
                    [:, :, 0:64],
                    pm[:, 0:TV].rearrange("p (pr x) -> p pr x", x=64),
                    W["bv_p_bc"][:].rearrange("p (pr x) -> p pr x", x=64),
                    ALU.add)
            return pqTp

        def _tail(c, pqTp):
            l0 = c * LC
            # pixel QK (row-packed strips, 2 heads per round) + tanh,
            # then AV per 4-head group (4 accumulating zero-padded MMs)
            pretT = trans.tile([P, 2, LC], F32R, tag="pretT", name="pretT")
            for g_ in range(2):
                pattn = trans.tile([P, 4, LC], F32R, tag="pattn", name="pattn")
                for r in range(2):
                    pq_ = qk_ps.tile([P, 2 * LC], F32, tag="qk", name="pq_")
                    for i in range(2):
                        h = 4 * g_ + 2 * r + i
                        pos = 32 * (h % 4)
                        nc.tensor.matmul(pq_[:, LC * i:LC * (i + 1)],
                                         mkpT[pos:pos + 32, h // 4, :],
                                         pqTp[pos:pos + 32, h // 4, :],
                                         start=True, stop=True,
                                         tile_position=(pos, 0))
                    nc.scalar.activation(
                        pattn[:, 2 * r:2 * r + 2, :]
                        .rearrange("p a b -> p (a b)"),
                        pq_[:], AF.Tanh, scale=0.5)
                pm = mm_ps.tile([P, LC], F32, tag="mm", name="pm")
                for i in range(4):
                    nc.tensor.matmul(pm[:], mvpx_pad[:, g_, i, :],
                                     pattn[:, i, :],
                                     start=(i == 0), stop=(i == 3))
                nc.vector.tensor_scalar(pretT[:, g_, :], pm[:],
                                        biaspret[:, g_:g_ + 1], 0.0,
                                        ALU.add, ALU.max)

            # conv3 + residual + relu (feature-major), then transpose out
            pm = mm_ps.tile([P, LC], F32, tag="mm", name="pm")
            for kc in range(2):
                nc.tensor.matmul(pm[:], W["w3p"][:, kc, :], pretT[:, kc, :],
                                 start=(kc == 0), stop=(kc == 1))
            poutT = trans.tile([P, LC], F32, tag="poutT", name="poutT")
            nc.vector.tensor_tensor(poutT[:], pm[:], xpT[:, l0:l0 + LC], ALU.add)
            nc.vector.tensor_scalar(poutT[:], poutT[:], W["b3p"][:], 0.0,
                                    ALU.add, ALU.max)
            pst2 = mm_ps.tile([P, LC], F32, tag="mm", name="pst2")
            for j in range(4):
                nc.tensor.transpose(pst2[:, P * j:P * (j + 1)],
                                    poutT[:, P * j:P * (j + 1)], ident[:])
            pout = trans.tile([P, LC], F32, tag="poutT", name="pout")
            nc.vector.tensor_copy(pout[:], pst2[:])
            for j in range(4):
                nc.sync.dma_start(ypix_d[l0 + P * j:l0 + P * (j + 1), :],
                                  pout[:, P * j:P * (j + 1)])

        # ---- memory attention m-tile (phase C body), interleavable ----
        avp_box = {}

        def _memtile(t):
            pqm = qk_ps.tile([P, 2 * LC], F32, tag="qk", name="pqm")
            for u in range(2):
                nc.tensor.matmul(pqm[:, LC * u:LC * (u + 1)],
                                 kcatT[:, P * t:P * (t + 1)],
                                 mq_bd[:, LC * u:LC * (u + 1)],
                                 start=True, stop=True)
            probs = trans3.tile([P, 2 * LC], F32R, tag="probs", name="probs")
            nc.scalar.activation(probs[:], pqm[:], AF.Exp)
            for pr in range(4):
                nc.tensor.matmul(
                    avp_box["a"][pr // 2]
                    [0:65, 256 * (pr % 2):256 * (pr % 2) + 256],
                    vaug[:, t, 65 * pr:65 * pr + 65],
                    probs[:, 256 * pr:256 * (pr + 1)],
                    start=(t == 0), stop=(t == MT - 1))

        pend = {}
        done_mt = 0
        for c in range(NCHUNK + 1):
            if c < NCHUNK:
                pend[c] = _front(c)
            if c >= 1:
                _tail(c - 1, pend.pop(c - 1))
            # interleave early memory-attention m-tiles under the last pixel
            # chunks (m-tile t needs only pixel chunk t//4's k/v outputs)
            if c >= 3:
                if "a" not in avp_box:
                    avp_box["a"] = [
                        mm_ps.tile([P, LC], F32, tag="mm", name=f"av{q}")
                        for q in range(2)]
                lim = min(MT, 6 * (c - 2))
                while done_mt < lim:
                    _memtile(done_mt)
                    done_mt += 1

        # ================= PHASE C: remaining memory attention ==========
        while done_mt < MT:
            _memtile(done_mt)
            done_mt += 1
        avp = avp_box["a"]

        # ================= PHASE D: memory tail =================
        # denominator rows: one pair at a time through a small row-64 buffer
        dsb = persist.tile([P, 512], F32, tag="dsb")
        for pr in range(4):
            q, s_ = divmod(pr, 2)
            col = 256 * (pr % 2)
            nc.vector.tensor_copy(dsb[64:65, col:col + 256],
                                  avp[q][64:65, 256 * s_:256 * (s_ + 1)])
            psd = qk_ps.tile([P, 2 * LC], F32, tag="qk")
            nc.tensor.matmul(psd[0:64, 0:256], ones64[64:65, :],
                             dsb[64:65, col:col + 256],
                             start=True, stop=True, tile_position=(64, 0))
            recip = trans3.tile([64, 256], F32, tag="recip")
            nc.vector.reciprocal(recip[:], psd[0:64, 0:256])
            ro = 64 * s_
            nc.vector.tensor_tensor(retT[ro:ro + 32, q, :],
                                    avp[q][0:32, 256 * s_:256 * s_ + P],
                                    recip[0:32, 0:P], ALU.mult)
            nc.vector.tensor_tensor(retT[ro + 32:ro + 64, q, :],
                                    avp[q][32:64, 256 * s_ + P:256 * (s_ + 1)],
                                    recip[32:64, P:2 * P], ALU.mult)
        for q in range(2):
            nc.vector.tensor_scalar(retT[:, q, :], retT[:, q, :],
                                    W["b_mret_col"][:, q:q + 1], 0.0,
                                    ALU.add, ALU.max)

        if dbg:
            for q in range(2):
                dd = trans.tile([P, LC], F32, tag="poutT", name="dd")[:, 0:P]
                nc.vector.tensor_copy(dd[:], retT[:, q, :])
                nc.sync.dma_start(dbg["d_ret"][:, q, :], dd[:])
                da = trans.tile([P, LC], F32, tag="poutT", name="da")
                nc.vector.tensor_copy(da[:], avp[q][:])
                nc.sync.dma_start(dbg["d_av"][q], da[:])
                dm1 = trans.tile([P, LC], F32, tag="poutT", name="dm1")[:, 0:P]
                nc.vector.tensor_copy(dm1[:], m1T[:, q, :])
                nc.sync.dma_start(dbg["d_m1"][:, q, :], dm1[:])
            nc.sync.dma_start(dbg["d_dsb"][:, 0:256], dsb[:, 0:256])

        # mem conv3 feature-major (+ residual + relu) -> moT directly
        pm = mm_ps.tile([P, LC], F32, tag="mm")
        for kc in range(2):
            nc.tensor.matmul(pm[:, 0:P], W["wm3"][:, kc, :], retT[:, kc, :],
                             start=(kc == 0), stop=(kc == 1))
        moT = trans.tile([P, P], F32, tag="moT")
        nc.vector.tensor_tensor(moT[:], pm[:, 0:P], xm_b3T[:], ALU.add)
        nc.vector.tensor_scalar(moT[:], moT[:], 0.0, None, ALU.max)
        moTr = trans.tile([P, P], F32R, tag="moTr")
        nc.vector.tensor_copy(moTr[:], moT[:])

        if dbg:
            dmo = trans.tile([P, LC], F32, tag="poutT", name="dmo")[:, 0:P]
            nc.vector.tensor_copy(dmo[:], mo[:])
            nc.sync.dma_start(dbg["d_mo"][:], dmo[:])

        # ffn1 -> ffnT [128, 16, 128]
        ffnT = persist.tile([P, 16, P], F32R, tag="ffnT")
        for g_ in range(4):
            pm = mm_ps.tile([P, LC], F32, tag="mm")
            for s_ in range(4):
                mc = 4 * g_ + s_
                nc.tensor.matmul(pm[:, P * s_:P * (s_ + 1)],
                                 W["wf1"][:, mc, :], moTr[:],
                                 start=True, stop=True)
            for s_ in range(4):
                mc = 4 * g_ + s_
                nc.scalar.activation(ffnT[:, mc, :],
                                     pm[:, P * s_:P * (s_ + 1)], AF.Relu,
                                     bias=W["bf1"][:, mc:mc + 1])
        if dbg:
            for mc in range(16):
                df = trans.tile([P, LC], F32, tag="poutT", name="df")[:, 0:P]
                nc.vector.tensor_copy(df[:], ffnT[:, mc, :])
                nc.sync.dma_start(dbg["d_ffn"][:, mc, :], df[:])

        # ffn2 (+ residual + relu) -> transpose -> ymem
        # two independent accumulators so the chain overlaps ffn1 production
        pma = mm_ps.tile([P, LC], F32, tag="mm", name="pma")
        pmb = mm_ps.tile([P, LC], F32, tag="mm", name="pmb")
        for kc in range(8):
            nc.tensor.matmul(pma[:, 0:P], W["wf2"][:, kc, :], ffnT[:, kc, :],
                             start=(kc == 0), stop=(kc == 7))
        for kc in range(8, 16):
            nc.tensor.matmul(pmb[:, 0:P], W["wf2"][:, kc, :], ffnT[:, kc, :],
                             start=(kc == 8), stop=(kc == 15))
        mo2T = trans.tile([P, P], F32, tag="mo2T")
        nc.vector.tensor_tensor(mo2T[:], pma[:, 0:P], moT[:], ALU.add)
        nc.vector.tensor_tensor(mo2T[:], pmb[:, 0:P], mo2T[:], ALU.add)
        nc.vector.tensor_scalar(mo2T[:], mo2T[:], W["bf2"][:], 0.0,
                                ALU.add, ALU.max)
        pst = mm_ps.tile([P, LC], F32, tag="mm")
        nc.tensor.transpose(pst[:, 0:P], mo2T[:], ident[:])
        mo2 = trans.tile([P, P], F32, tag="mo2")
        nc.vector.tensor_copy(mo2[:], pst[:, 0:P])
        nc.sync.dma_start(ymem_d[:], mo2[:])


def kernel(**inputs):
    if "nc" not in _cached:
        _cached["nc"] = _build_program()
    nc = _cached["nc"]
    d = _prep_host(inputs)
    pix = np.ascontiguousarray(np.asarray(inputs["pixel_input"], np.float32))
    mem = np.ascontiguousarray(np.asarray(inputs["memory_input"], np.float32))
    in_maps = []
    for b in range(B):
        m = {"xp": pix[b], "xm": mem[b]}
        m.update(d)
        in_maps.append(m)
    res = run_bass_kernel_spmd(nc, in_maps, core_ids=list(range(B)))
    pix_out = np.stack([res.results[b]["ypix"] for b in range(B)])
    mem_out = np.stack([res.results[b]["ymem"] for b in range(B)])
    return pix_out, mem_out


# revision 50
# speedup vs baseline: 1.0881x; 1.0077x over previous
"""Trainium2 Bass kernel for nn_DualPathTransformerLayer.

Sharding: data-parallel over batch -- B=8 batch elements, one per NeuronCore.
Each core runs an identical single-core program (SPMD) on its own batch slice;
weights are broadcast. No collectives needed.

Single-core program layout summary:
  - activations kept feature-major ("T" = [channels, positions]) where matmuls
    consume them, natural where needed (values for attention, outputs)
  - all BN affines folded into weights/bias host-side (inference BN)
  - pixel attention: sigmoid(x) = 0.5 + 0.5*tanh(x/2); the 0.5 factors and the
    BN-ret gamma are folded into the value projection; tanh on ACT engine
  - memory attention: softmax without max-subtraction (|logits| <= ~9);
    logits computed transposed [keys, (head, query)] via a block-diagonal
    query operand; denominator rides the AV matmul as an extra ones column
  - matmuls in float32r (1 cyc/row at N>=256) except tiny ones
"""
import numpy as np
import concourse.bass as bass
from concourse import bacc
import concourse.mybir as mybir
import concourse.tile as tile
from concourse.bass_utils import run_bass_kernel_spmd
from concourse.masks import make_identity

# problem dims (hardcoded per contract)
B, L, N, F = 8, 4096, 128, 128
H, DK, DV = 8, 16, 32
TK, TV = H * DK, H * DV          # 128, 256
BOT, FFN = 256, 2048
EPS = 1e-3
INV = float(1.0 / np.sqrt(1.0 + EPS))
P = 128
NCHUNK = 8          # pixel l-chunks of 512
LC = 512            # l-chunk size
MT = (L + N) // P   # 33 m-tiles for memory attention

F32 = mybir.dt.float32
F32R = mybir.dt.float32r
AF = mybir.ActivationFunctionType
ALU = mybir.AluOpType

_cached = {}
DEBUG = False


def _prep_host(w):
    """Fold BN affines into weights/biases. Returns dict of np arrays."""
    d = {}
    g = lambda p: np.asarray(p[0], np.float32) * INV
    be = lambda p: np.asarray(p[1], np.float32)

    def f32(x):
        return np.ascontiguousarray(x, np.float32)

    # ---- pixel conv1 ----
    g1, b1 = g(w["bn_pix1"]), be(w["bn_pix1"])
    W1 = np.asarray(w["W_pix1"], np.float32) * g1[None, :]          # [128,256]
    d["w1"] = f32(W1.reshape(F, 2, P))                              # lhsT [c, mc, m]
    d["b1"] = f32(b1.reshape(2, P).T)                               # [128, 2]

    # ---- pixel qkv ----
    gq, bq = g(w["bn_pix_qkv"]), be(w["bn_pix_qkv"])
    Wq = np.asarray(w["W_pix_qkv"], np.float32)
    Wqk = Wq * gq[None, :]
    bqk = bq.copy()
    # fold pixel-sim gamma into pixel q columns (per head)
    gs_pix = g(w["bn_pix_sim"])        # [H]
    bs_pix = be(w["bn_pix_sim"])       # [H]
    Wq_q = Wqk[:, :TK].copy()
    bq_q = bqk[:TK].copy()
    for h in range(H):
        Wq_q[:, h * DK:(h + 1) * DK] *= gs_pix[h]
        bq_q[h * DK:(h + 1) * DK] *= gs_pix[h]
    # pad q to 32-per-head; slot 16 is the constant-1 channel (bias-in-matmul)
    Wq_pad = np.zeros((BOT, H * 32), np.float32)
    bq_pad = np.zeros(H * 32, np.float32)
    for h in range(H):
        Wq_pad[:, h * 32:h * 32 + DK] = Wq_q[:, h * DK:(h + 1) * DK]
        bq_pad[h * 32:h * 32 + DK] = bq_q[h * DK:(h + 1) * DK]
        bq_pad[h * 32 + DK] = 1.0
    d["wq"] = f32(Wq_pad.reshape(2, P, 2, P).transpose(1, 0, 2, 3))  # [c,kc,mc,m]
    d["bq"] = f32(bq_pad.reshape(2, P).T)                            # [128,2]
    # k compact
    d["wk"] = f32(Wqk[:, TK:2 * TK].reshape(2, P, P).transpose(1, 0, 2))  # [c,kc,m]
    d["bk"] = f32(bqk[TK:2 * TK].reshape(P, 1))
    # v: fold mem-ret gamma (pv feeds only the memory-path v_cat)
    g_mret = g(w["bn_mem_ret"]).reshape(TV)      # [(h,dv)]
    b_mret = be(w["bn_mem_ret"]).reshape(TV)
    Wv_p = Wqk[:, 2 * TK:] * g_mret[None, :]
    bv_p = bqk[2 * TK:] * g_mret
    d["wv_p"] = f32(Wv_p.reshape(2, P, TV).transpose(1, 0, 2))       # [c,kc,n]
    d["bv_p_bc"] = f32(np.tile(bv_p[None, :], (P, 1)))               # [128,256]
    d["b_mret_col"] = f32(b_mret.reshape(2, P).T)                    # [128,2]

    # ---- memory conv1 ----
    gm1, bm1 = g(w["bn_mem1"]), be(w["bn_mem1"])
    Wm1 = np.asarray(w["W_mem1"], np.float32) * gm1[None, :]
    d["wm1"] = f32(Wm1.reshape(F, 2, P))
    d["bm1"] = f32(bm1.reshape(2, P).T)

    # ---- memory qkv ----
    gqm, bqm = g(w["bn_mem_qkv"]), be(w["bn_mem_qkv"])
    Wqm = np.asarray(w["W_mem_qkv"], np.float32) * gqm[None, :]
    bqm_f = bqm.copy()
    gs_mem = g(w["bn_mem_sim"])
    # (mem-sim beta cancels in softmax -- dropped)
    Wq_m = Wqm[:, :TK].copy()
    bq_m = bqm_f[:TK].copy()
    for h in range(H):
        Wq_m[:, h * DK:(h + 1) * DK] *= gs_mem[h]
        bq_m[h * DK:(h + 1) * DK] *= gs_mem[h]
    d["wqm"] = f32(Wq_m.reshape(2, P, P).transpose(1, 0, 2))
    d["bqm"] = f32(bq_m.reshape(P, 1))
    # mk compact (for mem-path k_cat)
    d["wkm"] = f32(Wqm[:, TK:2 * TK].reshape(2, P, P).transpose(1, 0, 2))
    d["bkm"] = f32(bqm_f[TK:2 * TK].reshape(P, 1))
    # mk padded (for pixel QK strips; slot 16 carries pixel-sim beta)
    Wk_m = Wqm[:, TK:2 * TK]
    Wk_pad = np.zeros((BOT, H * 32), np.float32)
    bk_pad = np.zeros(H * 32, np.float32)
    for h in range(H):
        Wk_pad[:, h * 32:h * 32 + DK] = Wk_m[:, h * DK:(h + 1) * DK]
        bk_pad[h * 32:h * 32 + DK] = bqm_f[TK + h * DK:TK + (h + 1) * DK]
        bk_pad[h * 32 + DK] = bs_pix[h]
    d["wkmp"] = f32(Wk_pad.reshape(2, P, 2, P).transpose(1, 0, 2, 3))
    d["bkmp"] = f32(bk_pad.reshape(2, P).T)
    # mv for v_cat (mem-ret gamma folded)
    Wv_m = Wqm[:, 2 * TK:]
    bv_m = bqm_f[2 * TK:]
    Wv_mc = Wv_m * g_mret[None, :]
    bv_mc = bv_m * g_mret
    d["wvm_c"] = f32(Wv_mc.reshape(2, P, TV).transpose(1, 0, 2))
    d["bvm_c_bc"] = f32(np.tile(bv_mc[None, :], (P, 1)))
    # mv for pixel AV (0.5 * pix-ret gamma folded)
    g_pret = g(w["bn_pix_ret"]).reshape(TV)
    b_pret = be(w["bn_pix_ret"]).reshape(TV)
    Wv_mp = Wv_m * (0.5 * g_pret)[None, :]
    bv_mp = bv_m * (0.5 * g_pret)
    d["wvm_p"] = f32(Wv_mp.reshape(2, P, TV).transpose(1, 0, 2))
    d["bvm_p_bc"] = f32(np.tile(bv_mp[None, :], (P, 1)))
    d["b_pret_col"] = f32(b_pret.reshape(2, P).T)                    # [128,2]

    # ---- conv3 ----
    g3, b3 = g(w["bn_pix3"]), be(w["bn_pix3"])
    W3 = np.asarray(w["W_pix3"], np.float32) * g3[None, :]           # [256,128]
    d["w3p"] = f32(W3.reshape(2, P, P).transpose(1, 0, 2))           # [c,kc,m]
    d["b3p"] = f32(b3.reshape(P, 1))
    g3m, b3m = g(w["bn_mem3"]), be(w["bn_mem3"])
    W3m = np.asarray(w["W_mem3"], np.float32) * g3m[None, :]
    d["wm3"] = f32(W3m.reshape(2, P, P).transpose(1, 0, 2))
    d["b3m_col"] = f32(b3m.reshape(P, 1))

    # ---- ffn ----
    gf1, bf1 = g(w["bn_ffn1"]), be(w["bn_ffn1"])
    Wf1 = np.asarray(w["W_ffn1"], np.float32) * gf1[None, :]         # [128,2048]
    d["wf1"] = f32(Wf1.reshape(F, 16, P))                            # lhsT [c,mc,m]
    d["bf1"] = f32(bf1.reshape(16, P).T)                             # [128,16]
    gf2, bf2 = g(w["bn_ffn2"]), be(w["bn_ffn2"])
    Wf2 = np.asarray(w["W_ffn2"], np.float32) * gf2[None, :]         # [2048,128]
    d["wf2"] = f32(Wf2.reshape(16, P, P).transpose(1, 0, 2))         # [c,kc,m]
    d["bf2"] = f32(bf2.reshape(P, 1))

    # concat everything into two blobs ([128, X] each) for 2 big DMAs
    rcols, fcols = [], []
    offs = {}
    for name, shape, dt_ in WEIGHT_SPECS:
        a = d[name].reshape(P, -1)
        tgt = rcols if dt_ == F32R else fcols
        off = sum(x.shape[1] for x in tgt)
        offs[name] = off
        tgt.append(a)
    out = {"wblob_r": np.ascontiguousarray(np.concatenate(rcols, axis=1)),
           "wblob_f": np.ascontiguousarray(np.concatenate(fcols, axis=1))}
    return out


WEIGHT_SPECS = [
    # phase A (memory front) first -- their DMA is staged ahead
    ("wm1", (P, 2, P), F32R), ("bm1", (P, 2), F32),
    ("wqm", (P, 2, P), F32R), ("bqm", (P, 1), F32),
    ("wkm", (P, 2, P), F32R), ("bkm", (P, 1), F32),
    ("wkmp", (P, 2, 2, P), F32R), ("bkmp", (P, 2), F32),
    ("wvm_c", (P, 2, TV), F32R), ("bvm_c_bc", (P, TV), F32),
    ("wvm_p", (P, 2, TV), F32R), ("bvm_p_bc", (P, TV), F32),
    ("b_pret_col", (P, 2), F32), ("b3m_col", (P, 1), F32),
    ("w1", (P, 2, P), F32R), ("b1", (P, 2), F32),
    ("wq", (P, 2, 2, P), F32R), ("bq", (P, 2), F32),
    ("wk", (P, 2, P), F32R), ("bk", (P, 1), F32),
    ("wv_p", (P, 2, TV), F32R), ("bv_p_bc", (P, TV), F32),
    # late weights
    ("b_mret_col", (P, 2), F32),
    ("w3p", (P, 2, P), F32R), ("b3p", (P, 1), F32),
    ("wm3", (P, 2, P), F32R),
    ("wf1", (P, 16, P), F32R), ("bf1", (P, 16), F32),
    ("wf2", (P, 16, P), F32R), ("bf2", (P, 1), F32),
]
A_WEIGHTS = ["wm1", "bm1", "wqm", "bqm", "wkm", "bkm", "wkmp", "bkmp",
             "wvm_c", "bvm_c_bc", "wvm_p", "bvm_p_bc", "b_pret_col", "b3m_col",
             "w1", "b1", "wq", "bq", "wk", "bk", "wv_p", "bv_p_bc"]




def _blob_layout():
    ro, fo = {}, {}
    rc = fc = 0
    for name, shape, dt_ in WEIGHT_SPECS:
        ncol = int(np.prod(shape[1:]))
        if dt_ == F32R:
            ro[name] = (rc, ncol)
            rc += ncol
        else:
            fo[name] = (fc, ncol)
            fc += ncol
    return ro, rc, fo, fc


def _build_program():
    nc = bacc.Bacc("TRN2", target_bir_lowering=False)
    xp_d = nc.dram_tensor("xp", (L, F), F32, kind="ExternalInput")
    xm_d = nc.dram_tensor("xm", (N, F), F32, kind="ExternalInput")
    ro, rc, fo, fc = _blob_layout()
    wd = {
        "wblob_r": nc.dram_tensor("wblob_r", (P, rc), F32R, kind="ExternalInput"),
        "wblob_f": nc.dram_tensor("wblob_f", (P, fc), F32, kind="ExternalInput"),
    }
    ypix_d = nc.dram_tensor("ypix", (L, F), F32, kind="ExternalOutput")
    ymem_d = nc.dram_tensor("ymem", (N, F), F32, kind="ExternalOutput")
    dbg = {}
    if DEBUG:
        for nm, shape in (("d_ret", (P, 2, P)), ("d_dsb", (P, 512)),
                          ("d_mo", (P, P)), ("d_ffn", (P, 16, P)),
                          ("d_av", (2, P, LC)), ("d_m1", (P, 2, P))):
            dbg[nm] = nc.dram_tensor(nm, shape, F32, kind="ExternalOutput")

    with tile.TileContext(nc) as tc:
        _emit(nc, tc, xp_d, xm_d, wd, ypix_d, ymem_d, dbg)
    nc.finalize()
    return nc


def _emit(nc, tc, xp_d, xm_d, wd, ypix_d, ymem_d, dbg=None):
    from contextlib import ExitStack
    ctx = ExitStack()
    with ctx:
        const = ctx.enter_context(tc.tile_pool(name="const", bufs=1))
        persist = ctx.enter_context(tc.tile_pool(name="persist", bufs=1))
        trans = ctx.enter_context(tc.tile_pool(name="trans", bufs=2))
        trans3 = ctx.enter_context(tc.tile_pool(name="trans3", bufs=2))
        deep3 = ctx.enter_context(tc.tile_pool(name="deep3", bufs=4))
        mm_ps = ctx.enter_context(tc.tile_pool(name="mm_ps", bufs=3, space="PSUM"))
        tin_ps = ctx.enter_context(tc.tile_pool(name="tin_ps", bufs=1, space="PSUM"))
        qk_ps = ctx.enter_context(tc.tile_pool(name="qk_ps", bufs=2, space="PSUM"))

        # ---------------- constants / weights in SBUF ----------------
        ro, rc, fo, fc = _blob_layout()
        blob_r = const.tile([P, rc], F32R, tag="blob_r", name="blob_r")
        blob_f = const.tile([P, fc], F32, tag="blob_f", name="blob_f")
        # inputs + first pixel chunks first (the DMA pipe is a FIFO), then
        # weights staged by first use
        xm_nat0 = persist.tile([P, P], F32, tag="xm_nat0", name="xm_nat0")
        nc.sync.dma_start(xm_nat0[:], xm_d[:])
        xp_pre = persist.tile([P, 8, P], F32, tag="xp_pre", name="xp_pre")
        nc.sync.dma_start(xp_pre[:],
                          xp_d[0:1024, :].rearrange("(t p) c -> p t c", p=P))
        actwarm = const.tile([P, 2], F32, tag="actwarm", name="actwarm")
        r0_ = ro["wm1"][0] + ro["wm1"][1]
        f0_ = fo["bm1"][0] + fo["bm1"][1]
        ra = max(off + n_ for off, n_ in (ro[k] for k in A_WEIGHTS if k in ro))
        fa = max(off + n_ for off, n_ in (fo[k] for k in A_WEIGHTS if k in fo))
        nc.sync.dma_start(blob_r[:, :r0_], wd["wblob_r"][:, :r0_])
        nc.sync.dma_start(blob_f[:, :f0_], wd["wblob_f"][:, :f0_])
        nc.sync.dma_start(blob_r[:, r0_:ra], wd["wblob_r"][:, r0_:ra])
        nc.sync.dma_start(blob_f[:, f0_:fa], wd["wblob_f"][:, f0_:fa])
        nc.sync.dma_start(blob_r[:, ra:], wd["wblob_r"][:, ra:])
        nc.sync.dma_start(blob_f[:, fa:], wd["wblob_f"][:, fa:])
        W = {}
        for name, shape, dt_ in WEIGHT_SPECS:
            if dt_ == F32R:
                off, ncol = ro[name]
                ap = blob_r[:, off:off + ncol]
            else:
                off, ncol = fo[name]
                ap = blob_f[:, off:off + ncol]
            if len(shape) == 3:
                ap = ap.rearrange("p (a b) -> p a b", b=shape[2])
            elif len(shape) == 4:
                ap = ap.rearrange("p (a b c) -> p a b c", b=shape[2], c=shape[3])
            W[name] = ap
        ident = const.tile([P, P], F32, tag="ident")
        make_identity(nc, ident[:])
        ones_r = const.tile([P, 2], F32R, tag="ones_r")
        nc.vector.memset(ones_r[:].bitcast(F32), 1.0)
        ones64 = const.tile([P, 64], F32, tag="ones64")
        nc.vector.memset(ones64[:], 1.0)
        nc.scalar.activation(actwarm[:], ones64[:, 0:2], AF.Tanh)

        # ---------------- persistent buffers ----------------
        xpT = persist.tile([P, L], F32R, tag="xpT")            # 16 KB/part
        kcatT = persist.tile([P, L + N], F32R, tag="kcatT")    # 16.5 KB
        vaug = persist.tile([P, MT, 260], F32R, tag="vaug")    # 33.4 KB
        mq_bd = persist.tile([P, H * P], F32R, tag="mq_bd")    # 4 KB
        mvpx_pad = persist.tile([P, 2, 4, P], F32R, tag="mvpx_pad")  # 4 KB
        biaspret = persist.tile([P, 2], F32, tag="biaspret")
        xm_b3T = persist.tile([P, P], F32, tag="xm_b3T")
        mkpT = persist.tile([P, 2, P], F32R, tag="mkpT")
        mvpx = persist.tile([P, TV], F32R, tag="mvpx")
        retT = persist.tile([P, 2, P], F32R, tag="retT")

        # ones columns of vaug (slot 64 of each 65-wide pair block)
        nc.vector.memset(
            vaug[:].rearrange("p t (pr c) -> p t pr c", c=65)[:, :, :, 64:65]
            .bitcast(F32), 1.0)

        # ================= PHASE A: memory front =================
        ps = mm_ps.tile([P, LC], F32, tag="mm")
        nc.tensor.transpose(ps[:, 0:P], xm_nat0[:], ident[:])
        xmT = trans.tile([P, P], F32R, tag="xmT")
        nc.vector.tensor_copy(xmT[:], ps[:, 0:P])
        nc.vector.tensor_scalar(xm_b3T[:], ps[:, 0:P], W["b3m_col"][:], None,
                                ALU.add)

        # M1T feature-major [2][128, 128]
        m1T = persist.tile([P, 2, P], F32R, tag="m1T")
        for mc in range(2):
            pm = mm_ps.tile([P, LC], F32, tag="mm")
            nc.tensor.matmul(pm[:, 0:P], W["wm1"][:, mc, :], xmT[:],
                             start=True, stop=True)
            nc.vector.tensor_scalar(m1T[:, mc, :], pm[:, 0:P],
                                    W["bm1"][:, mc:mc + 1], 0.0, ALU.add, ALU.max)

        # mqT compact -> mq_bd blockdiag
        pm = mm_ps.tile([P, LC], F32, tag="mm")
        for kc in range(2):
            nc.tensor.matmul(pm[:, 0:P], W["wqm"][:, kc, :], m1T[:, kc, :],
                             start=(kc == 0), stop=(kc == 1))
        mqT = trans.tile([P, P], F32R, tag="mqT")
        nc.vector.tensor_scalar(mqT[:], pm[:, 0:P], W["bqm"][:], None, ALU.add)
        nc.vector.memset(mq_bd[:].bitcast(F32), 0.0)
        for h in range(H):
            nc.sync.dma_start(mq_bd[h * DK:(h + 1) * DK, h * P:(h + 1) * P],
                              mqT[h * DK:(h + 1) * DK, :])

        # mkT compact -> kcatT tail
        pm = mm_ps.tile([P, LC], F32, tag="mm")
        for kc in range(2):
            nc.tensor.matmul(pm[:, 0:P], W["wkm"][:, kc, :], m1T[:, kc, :],
                             start=(kc == 0), stop=(kc == 1))
        nc.vector.tensor_scalar(kcatT[:, L:L + N], pm[:, 0:P], W["bkm"][:],
                                None, ALU.add)

        # mk padded (pixel QK lhsT)
        for mc in range(2):
            pm = mm_ps.tile([P, LC], F32, tag="mm")
            for kc in range(2):
                nc.tensor.matmul(pm[:, 0:P], W["wkmp"][:, kc, mc, :],
                                 m1T[:, kc, :], start=(kc == 0), stop=(kc == 1))
            nc.vector.tensor_scalar(mkpT[:, mc, :], pm[:, 0:P],
                                    W["bkmp"][:, mc:mc + 1], None, ALU.add)

        # mv for v_cat -> vaug chunk 32
        pm = mm_ps.tile([P, LC], F32, tag="mm")
        for kc in range(2):
            nc.tensor.matmul(pm[:, 0:TV], m1T[:, kc, :], W["wvm_c"][:, kc, :],
                             start=(kc == 0), stop=(kc == 1))
        nc.vector.tensor_tensor(
            vaug[:, 32].rearrange("p (pr c) -> p pr c", c=65)[:, :, 0:64],
            pm[:, 0:TV].rearrange("p (pr c) -> p pr c", c=64),
            W["bvm_c_bc"][:].rearrange("p (pr c) -> p pr c", c=64), ALU.add)

        # mv for pixel AV (scaled); plus zero-padded per-head variant
        pm = mm_ps.tile([P, LC], F32, tag="mm")
        for kc in range(2):
            nc.tensor.matmul(pm[:, 0:TV], m1T[:, kc, :], W["wvm_p"][:, kc, :],
                             start=(kc == 0), stop=(kc == 1))
        nc.vector.tensor_tensor(mvpx[:], pm[:, 0:TV], W["bvm_p_bc"][:], ALU.add)
        nc.vector.memset(mvpx_pad[:].bitcast(F32), 0.0)
        for h in range(H):
            g_, i_ = divmod(h, 4)
            nc.vector.tensor_copy(
                mvpx_pad[:, g_, i_, 32 * i_:32 * i_ + 32],
                mvpx[:, 32 * h:32 * h + 32])

        # colsum of mvpx (pixel AV bias) + b_pret
        pm = mm_ps.tile([P, LC], F32, tag="mm")
        for c_ in range(2):
            nc.tensor.matmul(pm[:, 2 * c_:2 * c_ + 2],
                             mvpx[:, P * c_:P * (c_ + 1)],
                             ones_r[:], start=True, stop=True)
        nc.vector.tensor_tensor(biaspret[:], pm[:, 0:4:2], W["b_pret_col"][:],
                                ALU.add)

        # ================= PHASE B: pixel pipeline =================
        # Software-pipelined emission: front(c+1) is emitted before tail(c) so
        # the scheduler can fill attention-phase gaps with next-chunk work.
        def _front(c):
            l0 = c * LC
            pst = tin_ps.tile([P, LC], F32, tag="tin", name="pst")
            for j in range(4):
                if c < 2:
                    xnat = xp_pre[:, 4 * c + j, :]
                else:
                    xt_ = trans3.tile([P, P], F32, tag="xnat", name="xnat")
                    nc.sync.dma_start(xt_[:], xp_d[l0 + P * j:l0 + P * (j + 1), :])
                    xnat = xt_[:]
                nc.tensor.transpose(pst[:, P * j:P * (j + 1)], xnat, ident[:])
            nc.vector.tensor_copy(xpT[:, l0:l0 + LC], pst[:])

            # conv1 -> P1T [2][128, 512]
            p1T = deep3.tile([P, 2, LC], F32R, tag="p1T", name="p1T")
            for mc in range(2):
                pm = tin_ps.tile([P, LC], F32, tag="tin", name="pm")
                nc.tensor.matmul(pm[:], W["w1"][:, mc, :],
                                 xpT[:, l0:l0 + LC],
                                 start=True, stop=True)
                nc.scalar.activation(p1T[:, mc, :], pm[:], AF.Relu,
                                     bias=W["b1"][:, mc:mc + 1])

            # qkv projections
            pqTp = deep3.tile([P, 2, LC], F32R, tag="pqTp", name="pqTp")
            for mc in range(2):
                pm = mm_ps.tile([P, LC], F32, tag="mm", name="pm")
                for kc in range(2):
                    nc.tensor.matmul(pm[:], W["wq"][:, kc, mc, :], p1T[:, kc, :],
                                     start=(kc == 0), stop=(kc == 1))
                nc.vector.tensor_scalar(pqTp[:, mc, :], pm[:],
                                        W["bq"][:, mc:mc + 1], None, ALU.add)
            pm = mm_ps.tile([P, LC], F32, tag="mm", name="pm")
            for kc in range(2):
                nc.tensor.matmul(pm[:], W["wk"][:, kc, :], p1T[:, kc, :],
                                 start=(kc == 0), stop=(kc == 1))
            nc.vector.tensor_scalar(kcatT[:, l0:l0 + LC], pm[:], W["bk"][:],
                                    None, ALU.add)
            for lt in range(4):
                pm = mm_ps.tile([P, LC], F32, tag="mm", name="pm")
                for kc in range(2):
                    nc.tensor.matmul(pm[:, 0:TV],
                                     p1T[:, kc, P * lt:P * (lt + 1)],
                                     W["wv_p"][:, kc, :],
                                     start=(kc == 0), stop=(kc == 1))
                nc.vector.tensor_tensor(
                    vaug[:, 4 * c + lt].rearrange("p (pr x) -> p pr x", x=65)
                    [:, :, 0:64],
                    pm[:, 0:TV].rearrange("p (pr x) -> p pr x", x=64),
                    W["bv_p_bc"][:].rearrange("p (pr x) -> p pr x", x=64),
                    ALU.add)
            return pqTp

        def _tail(c, pqTp):
            l0 = c * LC
            # pixel QK (row-packed strips, 2 heads per round) + tanh,
            # then AV per 4-head group (4 accumulating zero-padded MMs)
            pretT = trans.tile([P, 2, LC], F32R, tag="pretT", name="pretT")
            for g_ in range(2):
                pattn = trans.tile([P, 4, LC], F32R, tag="pattn", name="pattn")
                for r in range(2):
                    pq_ = qk_ps.tile([P, 2 * LC], F32, tag="qk", name="pq_")
                    for i in range(2):
                        h = 4 * g_ + 2 * r + i
                        pos = 32 * (h % 4)
                        nc.tensor.matmul(pq_[:, LC * i:LC * (i + 1)],
                                         mkpT[pos:pos + 32, h // 4, :],
                                         pqTp[pos:pos + 32, h // 4, :],
                                         start=True, stop=True,
                                         tile_position=(pos, 0))
                    nc.scalar.activation(
                        pattn[:, 2 * r:2 * r + 2, :]
                        .rearrange("p a b -> p (a b)"),
                        pq_[:], AF.Tanh, scale=0.5)
                pm = mm_ps.tile([P, LC], F32, tag="mm", name="pm")
                for i in range(4):
                    nc.tensor.matmul(pm[:], mvpx_pad[:, g_, i, :],
                                     pattn[:, i, :],
                                     start=(i == 0), stop=(i == 3))
                nc.vector.tensor_scalar(pretT[:, g_, :], pm[:],
                                        biaspret[:, g_:g_ + 1], 0.0,
                                        ALU.add, ALU.max)

            # conv3 + residual + relu (feature-major), then transpose out
            pm = mm_ps.tile([P, LC], F32, tag="mm", name="pm")
            for kc in range(2):
                nc.tensor.matmul(pm[:], W["w3p"][:, kc, :], pretT[:, kc, :],
                                 start=(kc == 0), stop=(kc == 1))
            poutT = trans.tile([P, LC], F32, tag="poutT", name="poutT")
            nc.vector.tensor_tensor(poutT[:], pm[:], xpT[:, l0:l0 + LC], ALU.add)
            nc.vector.tensor_scalar(poutT[:], poutT[:], W["b3p"][:], 0.0,
                                    ALU.add, ALU.max)
            pst2 = mm_ps.tile([P, LC], F32, tag="mm", name="pst2")
            for j in range(4):
                nc.tensor.transpose(pst2[:, P * j:P * (j + 1)],
                                    poutT[:, P * j:P * (j + 1)], ident[:])
            pout = trans.tile([P, LC], F32, tag="poutT", name="pout")
            nc.vector.tensor_copy(pout[:], pst2[:])
            for j in range(4):
                nc.sync.dma_start(ypix_d[l0 + P * j:l0 + P * (j + 1), :],
                                  pout[:, P * j:P * (j + 1)])

        # ---- memory attention m-tile (phase C body), interleavable ----
        avp_box = {}

        def _memtile(t):
            pqm = qk_ps.tile([P, 2 * LC], F32, tag="qk", name="pqm")
            for u in range(2):
                nc.tensor.matmul(pqm[:, LC * u:LC * (u + 1)],
                                 kcatT[:, P * t:P * (t + 1)],
                                 mq_bd[:, LC * u:LC * (u + 1)],
                                 start=True, stop=True)
            probs = trans3.tile([P, 2 * LC], F32R, tag="probs", name="probs")
            nc.scalar.activation(probs[:], pqm[:], AF.Exp)
            for pr in range(4):
                nc.tensor.matmul(
                    avp_box["a"][pr // 2]
                    [0:65, 256 * (pr % 2):256 * (pr % 2) + 256],
                    vaug[:, t, 65 * pr:65 * pr + 65],
                    probs[:, 256 * pr:256 * (pr + 1)],
                    start=(t == 0), stop=(t == MT - 1))

        pend = {}
        done_mt = 0
        for c in range(NCHUNK + 1):
            if c < NCHUNK:
                pend[c] = _front(c)
            if c >= 1:
                _tail(c - 1, pend.pop(c - 1))
            # interleave early memory-attention m-tiles under the last pixel
            # chunks (m-tile t needs only pixel chunk t//4's k/v outputs)
            if c >= 3:
                if "a" not in avp_box:
                    avp_box["a"] = [
                        mm_ps.tile([P, LC], F32, tag="mm", name=f"av{q}")
                        for q in range(2)]
                lim = min(MT, 7 * (c - 2))
                while done_mt < lim:
                    _memtile(done_mt)
                    done_mt += 1

        # ================= PHASE C: remaining memory attention ==========
        while done_mt < MT:
            _memtile(done_mt)
            done_mt += 1
        avp = avp_box["a"]

        # ================= PHASE D: memory tail =================
        # denominator rows: one pair at a time through a small row-64 buffer
        dsb = persist.tile([P, 512], F32, tag="dsb")
        for pr in range(4):
            q, s_ = divmod(pr, 2)
            col = 256 * (pr % 2)
            nc.vector.tensor_copy(dsb[64:65, col:col + 256],
                                  avp[q][64:65, 256 * s_:256 * (s_ + 1)])
            psd = qk_ps.tile([P, 2 * LC], F32, tag="qk")
            nc.tensor.matmul(psd[0:64, 0:256], ones64[64:65, :],
                             dsb[64:65, col:col + 256],
                             start=True, stop=True, tile_position=(64, 0))
            recip = trans3.tile([64, 256], F32, tag="recip")
            nc.vector.reciprocal(recip[:], psd[0:64, 0:256])
            ro = 64 * s_
            nc.vector.tensor_tensor(retT[ro:ro + 32, q, :],
                                    avp[q][0:32, 256 * s_:256 * s_ + P],
                                    recip[0:32, 0:P], ALU.mult)
            nc.vector.tensor_tensor(retT[ro + 32:ro + 64, q, :],
                                    avp[q][32:64, 256 * s_ + P:256 * (s_ + 1)],
                                    recip[32:64, P:2 * P], ALU.mult)
        for q in range(2):
            nc.vector.tensor_scalar(retT[:, q, :], retT[:, q, :],
                                    W["b_mret_col"][:, q:q + 1], 0.0,
                                    ALU.add, ALU.max)

        if dbg:
            for q in range(2):
                dd = trans.tile([P, LC], F32, tag="poutT", name="dd")[:, 0:P]
                nc.vector.tensor_copy(dd[:], retT[:, q, :])
                nc.sync.dma_start(dbg["d_ret"][:, q, :], dd[:])
                da = trans.tile([P, LC], F32, tag="poutT", name="da")
                nc.vector.tensor_copy(da[:], avp[q][:])
                nc.sync.dma_start(dbg["d_av"][q], da[:])
                dm1 = trans.tile([P, LC], F32, tag="poutT", name="dm1")[:, 0:P]
                nc.vector.tensor_copy(dm1[:], m1T[:, q, :])
                nc.sync.dma_start(dbg["d_m1"][:, q, :], dm1[:])
            nc.sync.dma_start(dbg["d_dsb"][:, 0:256], dsb[:, 0:256])

        # mem conv3 feature-major (+ residual + relu) -> moT directly
        pm = mm_ps.tile([P, LC], F32, tag="mm")
        for kc in range(2):
            nc.tensor.matmul(pm[:, 0:P], W["wm3"][:, kc, :], retT[:, kc, :],
                             start=(kc == 0), stop=(kc == 1))
        moT = trans.tile([P, P], F32, tag="moT")
        nc.vector.tensor_tensor(moT[:], pm[:, 0:P], xm_b3T[:], ALU.add)
        nc.vector.tensor_scalar(moT[:], moT[:], 0.0, None, ALU.max)
        moTr = trans.tile([P, P], F32R, tag="moTr")
        nc.vector.tensor_copy(moTr[:], moT[:])

        if dbg:
            dmo = trans.tile([P, LC], F32, tag="poutT", name="dmo")[:, 0:P]
            nc.vector.tensor_copy(dmo[:], mo[:])
            nc.sync.dma_start(dbg["d_mo"][:], dmo[:])

        # ffn1 -> ffnT [128, 16, 128]
        ffnT = persist.tile([P, 16, P], F32R, tag="ffnT")
        for g_ in range(4):
            pm = mm_ps.tile([P, LC], F32, tag="mm")
            for s_ in range(4):
                mc = 4 * g_ + s_
                nc.tensor.matmul(pm[:, P * s_:P * (s_ + 1)],
                                 W["wf1"][:, mc, :], moTr[:],
                                 start=True, stop=True)
            for s_ in range(4):
                mc = 4 * g_ + s_
                nc.scalar.activation(ffnT[:, mc, :],
                                     pm[:, P * s_:P * (s_ + 1)], AF.Relu,
                                     bias=W["bf1"][:, mc:mc + 1])
        if dbg:
            for mc in range(16):
                df = trans.tile([P, LC], F32, tag="poutT", name="df")[:, 0:P]
                nc.vector.tensor_copy(df[:], ffnT[:, mc, :])
                nc.sync.dma_start(dbg["d_ffn"][:, mc, :], df[:])

        # ffn2 (+ residual + relu) -> transpose -> ymem
        # two independent accumulators so the chain overlaps ffn1 production
        pma = mm_ps.tile([P, LC], F32, tag="mm", name="pma")
        pmb = mm_ps.tile([P, LC], F32, tag="mm", name="pmb")
        for kc in range(8):
            nc.tensor.matmul(pma[:, 0:P], W["wf2"][:, kc, :], ffnT[:, kc, :],
                             start=(kc == 0), stop=(kc == 7))
        for kc in range(8, 16):
            nc.tensor.matmul(pmb[:, 0:P], W["wf2"][:, kc, :], ffnT[:, kc, :],
                             start=(kc == 8), stop=(kc == 15))
        mo2T = trans.tile([P, P], F32, tag="mo2T")
        nc.vector.tensor_tensor(mo2T[:], pma[:, 0:P], moT[:], ALU.add)
        nc.vector.tensor_tensor(mo2T[:], pmb[:, 0:P], mo2T[:], ALU.add)
        nc.vector.tensor_scalar(mo2T[:], mo2T[:], W["bf2"][:], 0.0,
                                ALU.add, ALU.max)
        pst = mm_ps.tile([P, LC], F32, tag="mm")
        nc.tensor.transpose(pst[:, 0:P], mo2T[:], ident[:])
        mo2 = trans.tile([P, P], F32, tag="mo2")
        nc.vector.tensor_copy(mo2[:], pst[:, 0:P])
        nc.sync.dma_start(ymem_d[:], mo2[:])


def kernel(**inputs):
    if "nc" not in _cached:
        _cached["nc"] = _build_program()
    nc = _cached["nc"]
    d = _prep_host(inputs)
    pix = np.ascontiguousarray(np.asarray(inputs["pixel_input"], np.float32))
    mem = np.ascontiguousarray(np.asarray(inputs["memory_input"], np.float32))
    in_maps = []
    for b in range(B):
        m = {"xp": pix[b], "xm": mem[b]}
        m.update(d)
        in_maps.append(m)
    res = run_bass_kernel_spmd(nc, in_maps, core_ids=list(range(B)))
    pix_out = np.stack([res.results[b]["ypix"] for b in range(B)])
    mem_out = np.stack([res.results[b]["ymem"] for b in range(B)])
    return pix_out, mem_out


# revision 51
# speedup vs baseline: 1.0888x; 1.0007x over previous
"""Trainium2 Bass kernel for nn_DualPathTransformerLayer.

Sharding: data-parallel over batch -- B=8 batch elements, one per NeuronCore.
Each core runs an identical single-core program (SPMD) on its own batch slice;
weights are broadcast. No collectives needed.

Single-core program layout summary:
  - activations kept feature-major ("T" = [channels, positions]) where matmuls
    consume them, natural where needed (values for attention, outputs)
  - all BN affines folded into weights/bias host-side (inference BN)
  - pixel attention: sigmoid(x) = 0.5 + 0.5*tanh(x/2); the 0.5 factors and the
    BN-ret gamma are folded into the value projection; tanh on ACT engine
  - memory attention: softmax without max-subtraction (|logits| <= ~9);
    logits computed transposed [keys, (head, query)] via a block-diagonal
    query operand; denominator rides the AV matmul as an extra ones column
  - matmuls in float32r (1 cyc/row at N>=256) except tiny ones
"""
import numpy as np
import concourse.bass as bass
from concourse import bacc
import concourse.mybir as mybir
import concourse.tile as tile
from concourse.bass_utils import run_bass_kernel_spmd
from concourse.masks import make_identity

# problem dims (hardcoded per contract)
B, L, N, F = 8, 4096, 128, 128
H, DK, DV = 8, 16, 32
TK, TV = H * DK, H * DV          # 128, 256
BOT, FFN = 256, 2048
EPS = 1e-3
INV = float(1.0 / np.sqrt(1.0 + EPS))
P = 128
NCHUNK = 8          # pixel l-chunks of 512
LC = 512            # l-chunk size
MT = (L + N) // P   # 33 m-tiles for memory attention

F32 = mybir.dt.float32
F32R = mybir.dt.float32r
AF = mybir.ActivationFunctionType
ALU = mybir.AluOpType

_cached = {}
DEBUG = False


def _prep_host(w):
    """Fold BN affines into weights/biases. Returns dict of np arrays."""
    d = {}
    g = lambda p: np.asarray(p[0], np.float32) * INV
    be = lambda p: np.asarray(p[1], np.float32)

    def f32(x):
        return np.ascontiguousarray(x, np.float32)

    # ---- pixel conv1 ----
    g1, b1 = g(w["bn_pix1"]), be(w["bn_pix1"])
    W1 = np.asarray(w["W_pix1"], np.float32) * g1[None, :]          # [128,256]
    d["w1"] = f32(W1.reshape(F, 2, P))                              # lhsT [c, mc, m]
    d["b1"] = f32(b1.reshape(2, P).T)                               # [128, 2]

    # ---- pixel qkv ----
    gq, bq = g(w["bn_pix_qkv"]), be(w["bn_pix_qkv"])
    Wq = np.asarray(w["W_pix_qkv"], np.float32)
    Wqk = Wq * gq[None, :]
    bqk = bq.copy()
    # fold pixel-sim gamma into pixel q columns (per head)
    gs_pix = g(w["bn_pix_sim"])        # [H]
    bs_pix = be(w["bn_pix_sim"])       # [H]
    Wq_q = Wqk[:, :TK].copy()
    bq_q = bqk[:TK].copy()
    for h in range(H):
        Wq_q[:, h * DK:(h + 1) * DK] *= gs_pix[h]
        bq_q[h * DK:(h + 1) * DK] *= gs_pix[h]
    # pad q to 32-per-head; slot 16 is the constant-1 channel (bias-in-matmul)
    Wq_pad = np.zeros((BOT, H * 32), np.float32)
    bq_pad = np.zeros(H * 32, np.float32)
    for h in range(H):
        Wq_pad[:, h * 32:h * 32 + DK] = Wq_q[:, h * DK:(h + 1) * DK]
        bq_pad[h * 32:h * 32 + DK] = bq_q[h * DK:(h + 1) * DK]
        bq_pad[h * 32 + DK] = 1.0
    d["wq"] = f32(Wq_pad.reshape(2, P, 2, P).transpose(1, 0, 2, 3))  # [c,kc,mc,m]
    d["bq"] = f32(bq_pad.reshape(2, P).T)                            # [128,2]
    # k compact
    d["wk"] = f32(Wqk[:, TK:2 * TK].reshape(2, P, P).transpose(1, 0, 2))  # [c,kc,m]
    d["bk"] = f32(bqk[TK:2 * TK].reshape(P, 1))
    # v: fold mem-ret gamma (pv feeds only the memory-path v_cat)
    g_mret = g(w["bn_mem_ret"]).reshape(TV)      # [(h,dv)]
    b_mret = be(w["bn_mem_ret"]).reshape(TV)
    Wv_p = Wqk[:, 2 * TK:] * g_mret[None, :]
    bv_p = bqk[2 * TK:] * g_mret
    d["wv_p"] = f32(Wv_p.reshape(2, P, TV).transpose(1, 0, 2))       # [c,kc,n]
    d["bv_p_bc"] = f32(np.tile(bv_p[None, :], (P, 1)))               # [128,256]
    d["b_mret_col"] = f32(b_mret.reshape(2, P).T)                    # [128,2]

    # ---- memory conv1 ----
    gm1, bm1 = g(w["bn_mem1"]), be(w["bn_mem1"])
    Wm1 = np.asarray(w["W_mem1"], np.float32) * gm1[None, :]
    d["wm1"] = f32(Wm1.reshape(F, 2, P))
    d["bm1"] = f32(bm1.reshape(2, P).T)

    # ---- memory qkv ----
    gqm, bqm = g(w["bn_mem_qkv"]), be(w["bn_mem_qkv"])
    Wqm = np.asarray(w["W_mem_qkv"], np.float32) * gqm[None, :]
    bqm_f = bqm.copy()
    gs_mem = g(w["bn_mem_sim"])
    # (mem-sim beta cancels in softmax -- dropped)
    Wq_m = Wqm[:, :TK].copy()
    bq_m = bqm_f[:TK].copy()
    for h in range(H):
        Wq_m[:, h * DK:(h + 1) * DK] *= gs_mem[h]
        bq_m[h * DK:(h + 1) * DK] *= gs_mem[h]
    d["wqm"] = f32(Wq_m.reshape(2, P, P).transpose(1, 0, 2))
    d["bqm"] = f32(bq_m.reshape(P, 1))
    # mk compact (for mem-path k_cat)
    d["wkm"] = f32(Wqm[:, TK:2 * TK].reshape(2, P, P).transpose(1, 0, 2))
    d["bkm"] = f32(bqm_f[TK:2 * TK].reshape(P, 1))
    # mk padded (for pixel QK strips; slot 16 carries pixel-sim beta)
    Wk_m = Wqm[:, TK:2 * TK]
    Wk_pad = np.zeros((BOT, H * 32), np.float32)
    bk_pad = np.zeros(H * 32, np.float32)
    for h in range(H):
        Wk_pad[:, h * 32:h * 32 + DK] = Wk_m[:, h * DK:(h + 1) * DK]
        bk_pad[h * 32:h * 32 + DK] = bqm_f[TK + h * DK:TK + (h + 1) * DK]
        bk_pad[h * 32 + DK] = bs_pix[h]
    d["wkmp"] = f32(Wk_pad.reshape(2, P, 2, P).transpose(1, 0, 2, 3))
    d["bkmp"] = f32(bk_pad.reshape(2, P).T)
    # mv for v_cat (mem-ret gamma folded)
    Wv_m = Wqm[:, 2 * TK:]
    bv_m = bqm_f[2 * TK:]
    Wv_mc = Wv_m * g_mret[None, :]
    bv_mc = bv_m * g_mret
    d["wvm_c"] = f32(Wv_mc.reshape(2, P, TV).transpose(1, 0, 2))
    d["bvm_c_bc"] = f32(np.tile(bv_mc[None, :], (P, 1)))
    # mv for pixel AV (0.5 * pix-ret gamma folded)
    g_pret = g(w["bn_pix_ret"]).reshape(TV)
    b_pret = be(w["bn_pix_ret"]).reshape(TV)
    Wv_mp = Wv_m * (0.5 * g_pret)[None, :]
    bv_mp = bv_m * (0.5 * g_pret)
    d["wvm_p"] = f32(Wv_mp.reshape(2, P, TV).transpose(1, 0, 2))
    d["bvm_p_bc"] = f32(np.tile(bv_mp[None, :], (P, 1)))
    d["b_pret_col"] = f32(b_pret.reshape(2, P).T)                    # [128,2]

    # ---- conv3 ----
    g3, b3 = g(w["bn_pix3"]), be(w["bn_pix3"])
    W3 = np.asarray(w["W_pix3"], np.float32) * g3[None, :]           # [256,128]
    d["w3p"] = f32(W3.reshape(2, P, P).transpose(1, 0, 2))           # [c,kc,m]
    d["b3p"] = f32(b3.reshape(P, 1))
    g3m, b3m = g(w["bn_mem3"]), be(w["bn_mem3"])
    W3m = np.asarray(w["W_mem3"], np.float32) * g3m[None, :]
    d["wm3"] = f32(W3m.reshape(2, P, P).transpose(1, 0, 2))
    d["b3m_col"] = f32(b3m.reshape(P, 1))

    # ---- ffn ----
    gf1, bf1 = g(w["bn_ffn1"]), be(w["bn_ffn1"])
    Wf1 = np.asarray(w["W_ffn1"], np.float32) * gf1[None, :]         # [128,2048]
    d["wf1"] = f32(Wf1.reshape(F, 16, P))                            # lhsT [c,mc,m]
    d["bf1"] = f32(bf1.reshape(16, P).T)                             # [128,16]
    gf2, bf2 = g(w["bn_ffn2"]), be(w["bn_ffn2"])
    Wf2 = np.asarray(w["W_ffn2"], np.float32) * gf2[None, :]         # [2048,128]
    d["wf2"] = f32(Wf2.reshape(16, P, P).transpose(1, 0, 2))         # [c,kc,m]
    d["bf2"] = f32(bf2.reshape(P, 1))

    # concat everything into two blobs ([128, X] each) for 2 big DMAs
    rcols, fcols = [], []
    offs = {}
    for name, shape, dt_ in WEIGHT_SPECS:
        a = d[name].reshape(P, -1)
        tgt = rcols if dt_ == F32R else fcols
        off = sum(x.shape[1] for x in tgt)
        offs[name] = off
        tgt.append(a)
    out = {"wblob_r": np.ascontiguousarray(np.concatenate(rcols, axis=1)),
           "wblob_f": np.ascontiguousarray(np.concatenate(fcols, axis=1))}
    return out


WEIGHT_SPECS = [
    # phase A (memory front) first -- their DMA is staged ahead
    ("wm1", (P, 2, P), F32R), ("bm1", (P, 2), F32),
    ("wqm", (P, 2, P), F32R), ("bqm", (P, 1), F32),
    ("wkm", (P, 2, P), F32R), ("bkm", (P, 1), F32),
    ("wkmp", (P, 2, 2, P), F32R), ("bkmp", (P, 2), F32),
    ("wvm_c", (P, 2, TV), F32R), ("bvm_c_bc", (P, TV), F32),
    ("wvm_p", (P, 2, TV), F32R), ("bvm_p_bc", (P, TV), F32),
    ("b_pret_col", (P, 2), F32), ("b3m_col", (P, 1), F32),
    ("w1", (P, 2, P), F32R), ("b1", (P, 2), F32),
    ("wq", (P, 2, 2, P), F32R), ("bq", (P, 2), F32),
    ("wk", (P, 2, P), F32R), ("bk", (P, 1), F32),
    ("wv_p", (P, 2, TV), F32R), ("bv_p_bc", (P, TV), F32),
    # late weights
    ("b_mret_col", (P, 2), F32),
    ("w3p", (P, 2, P), F32R), ("b3p", (P, 1), F32),
    ("wm3", (P, 2, P), F32R),
    ("wf1", (P, 16, P), F32R), ("bf1", (P, 16), F32),
    ("wf2", (P, 16, P), F32R), ("bf2", (P, 1), F32),
]
A_WEIGHTS = ["wm1", "bm1", "wqm", "bqm", "wkm", "bkm", "wkmp", "bkmp",
             "wvm_c", "bvm_c_bc", "wvm_p", "bvm_p_bc", "b_pret_col", "b3m_col",
             "w1", "b1", "wq", "bq", "wk", "bk", "wv_p", "bv_p_bc"]




def _blob_layout():
    ro, fo = {}, {}
    rc = fc = 0
    for name, shape, dt_ in WEIGHT_SPECS:
        ncol = int(np.prod(shape[1:]))
        if dt_ == F32R:
            ro[name] = (rc, ncol)
            rc += ncol
        else:
            fo[name] = (fc, ncol)
            fc += ncol
    return ro, rc, fo, fc


def _build_program():
    nc = bacc.Bacc("TRN2", target_bir_lowering=False)
    xp_d = nc.dram_tensor("xp", (L, F), F32, kind="ExternalInput")
    xm_d = nc.dram_tensor("xm", (N, F), F32, kind="ExternalInput")
    ro, rc, fo, fc = _blob_layout()
    wd = {
        "wblob_r": nc.dram_tensor("wblob_r", (P, rc), F32R, kind="ExternalInput"),
        "wblob_f": nc.dram_tensor("wblob_f", (P, fc), F32, kind="ExternalInput"),
    }
    ypix_d = nc.dram_tensor("ypix", (L, F), F32, kind="ExternalOutput")
    ymem_d = nc.dram_tensor("ymem", (N, F), F32, kind="ExternalOutput")
    dbg = {}
    if DEBUG:
        for nm, shape in (("d_ret", (P, 2, P)), ("d_dsb", (P, 512)),
                          ("d_mo", (P, P)), ("d_ffn", (P, 16, P)),
                          ("d_av", (2, P, LC)), ("d_m1", (P, 2, P))):
            dbg[nm] = nc.dram_tensor(nm, shape, F32, kind="ExternalOutput")

    with tile.TileContext(nc) as tc:
        _emit(nc, tc, xp_d, xm_d, wd, ypix_d, ymem_d, dbg)
    nc.finalize()
    return nc


def _emit(nc, tc, xp_d, xm_d, wd, ypix_d, ymem_d, dbg=None):
    from contextlib import ExitStack
    ctx = ExitStack()
    with ctx:
        const = ctx.enter_context(tc.tile_pool(name="const", bufs=1))
        persist = ctx.enter_context(tc.tile_pool(name="persist", bufs=1))
        trans = ctx.enter_context(tc.tile_pool(name="trans", bufs=2))
        trans3 = ctx.enter_context(tc.tile_pool(name="trans3", bufs=2))
        deep3 = ctx.enter_context(tc.tile_pool(name="deep3", bufs=4))
        mm_ps = ctx.enter_context(tc.tile_pool(name="mm_ps", bufs=3, space="PSUM"))
        tin_ps = ctx.enter_context(tc.tile_pool(name="tin_ps", bufs=1, space="PSUM"))
        qk_ps = ctx.enter_context(tc.tile_pool(name="qk_ps", bufs=2, space="PSUM"))

        # ---------------- constants / weights in SBUF ----------------
        ro, rc, fo, fc = _blob_layout()
        blob_r = const.tile([P, rc], F32R, tag="blob_r", name="blob_r")
        blob_f = const.tile([P, fc], F32, tag="blob_f", name="blob_f")
        # inputs + first pixel chunks first (the DMA pipe is a FIFO), then
        # weights staged by first use
        xm_nat0 = persist.tile([P, P], F32, tag="xm_nat0", name="xm_nat0")
        nc.sync.dma_start(xm_nat0[:], xm_d[:])
        xp_pre = persist.tile([P, 8, P], F32, tag="xp_pre", name="xp_pre")
        nc.sync.dma_start(xp_pre[:],
                          xp_d[0:1024, :].rearrange("(t p) c -> p t c", p=P))
        actwarm = const.tile([P, 2], F32, tag="actwarm", name="actwarm")
        r0_ = ro["wm1"][0] + ro["wm1"][1]
        f0_ = fo["bm1"][0] + fo["bm1"][1]
        ra = max(off + n_ for off, n_ in (ro[k] for k in A_WEIGHTS if k in ro))
        fa = max(off + n_ for off, n_ in (fo[k] for k in A_WEIGHTS if k in fo))
        nc.sync.dma_start(blob_r[:, :r0_], wd["wblob_r"][:, :r0_])
        nc.sync.dma_start(blob_f[:, :f0_], wd["wblob_f"][:, :f0_])
        nc.sync.dma_start(blob_r[:, r0_:ra], wd["wblob_r"][:, r0_:ra])
        nc.sync.dma_start(blob_f[:, f0_:fa], wd["wblob_f"][:, f0_:fa])
        nc.sync.dma_start(blob_r[:, ra:], wd["wblob_r"][:, ra:])
        nc.sync.dma_start(blob_f[:, fa:], wd["wblob_f"][:, fa:])
        W = {}
        for name, shape, dt_ in WEIGHT_SPECS:
            if dt_ == F32R:
                off, ncol = ro[name]
                ap = blob_r[:, off:off + ncol]
            else:
                off, ncol = fo[name]
                ap = blob_f[:, off:off + ncol]
            if len(shape) == 3:
                ap = ap.rearrange("p (a b) -> p a b", b=shape[2])
            elif len(shape) == 4:
                ap = ap.rearrange("p (a b c) -> p a b c", b=shape[2], c=shape[3])
            W[name] = ap
        ident = const.tile([P, P], F32, tag="ident")
        make_identity(nc, ident[:])
        ones_r = const.tile([P, 2], F32R, tag="ones_r")
        nc.vector.memset(ones_r[:].bitcast(F32), 1.0)
        ones64 = const.tile([P, 64], F32, tag="ones64")
        nc.vector.memset(ones64[:], 1.0)
        nc.scalar.activation(actwarm[:], ones64[:, 0:2], AF.Tanh)

        # ---------------- persistent buffers ----------------
        xpT = persist.tile([P, L], F32R, tag="xpT")            # 16 KB/part
        kcatT = persist.tile([P, L + N], F32R, tag="kcatT")    # 16.5 KB
        vaug = persist.tile([P, MT, 260], F32R, tag="vaug")    # 33.4 KB
        mq_bd = persist.tile([P, H * P], F32R, tag="mq_bd")    # 4 KB
        mvpx_pad = persist.tile([P, 2, 4, P], F32R, tag="mvpx_pad")  # 4 KB
        biaspret = persist.tile([P, 2], F32, tag="biaspret")
        xm_b3T = persist.tile([P, P], F32, tag="xm_b3T")
        mkpT = persist.tile([P, 2, P], F32R, tag="mkpT")
        mvpx = persist.tile([P, TV], F32R, tag="mvpx")
        retT = persist.tile([P, 2, P], F32R, tag="retT")

        # ones columns of vaug (slot 64 of each 65-wide pair block)
        nc.vector.memset(
            vaug[:].rearrange("p t (pr c) -> p t pr c", c=65)[:, :, :, 64:65]
            .bitcast(F32), 1.0)

        # ================= PHASE A: memory front =================
        ps = mm_ps.tile([P, LC], F32, tag="mm")
        nc.tensor.transpose(ps[:, 0:P], xm_nat0[:], ident[:])
        xmT = trans.tile([P, P], F32R, tag="xmT")
        nc.vector.tensor_copy(xmT[:], ps[:, 0:P])
        nc.vector.tensor_scalar(xm_b3T[:], ps[:, 0:P], W["b3m_col"][:], None,
                                ALU.add)

        # M1T feature-major [2][128, 128]
        m1T = persist.tile([P, 2, P], F32R, tag="m1T")
        for mc in range(2):
            pm = mm_ps.tile([P, LC], F32, tag="mm")
            nc.tensor.matmul(pm[:, 0:P], W["wm1"][:, mc, :], xmT[:],
                             start=True, stop=True)
            nc.vector.tensor_scalar(m1T[:, mc, :], pm[:, 0:P],
                                    W["bm1"][:, mc:mc + 1], 0.0, ALU.add, ALU.max)

        # mqT compact -> mq_bd blockdiag
        pm = mm_ps.tile([P, LC], F32, tag="mm")
        for kc in range(2):
            nc.tensor.matmul(pm[:, 0:P], W["wqm"][:, kc, :], m1T[:, kc, :],
                             start=(kc == 0), stop=(kc == 1))
        mqT = trans.tile([P, P], F32R, tag="mqT")
        nc.vector.tensor_scalar(mqT[:], pm[:, 0:P], W["bqm"][:], None, ALU.add)
        nc.vector.memset(mq_bd[:].bitcast(F32), 0.0)
        for h in range(H):
            nc.sync.dma_start(mq_bd[h * DK:(h + 1) * DK, h * P:(h + 1) * P],
                              mqT[h * DK:(h + 1) * DK, :])

        # mkT compact -> kcatT tail
        pm = mm_ps.tile([P, LC], F32, tag="mm")
        for kc in range(2):
            nc.tensor.matmul(pm[:, 0:P], W["wkm"][:, kc, :], m1T[:, kc, :],
                             start=(kc == 0), stop=(kc == 1))
        nc.vector.tensor_scalar(kcatT[:, L:L + N], pm[:, 0:P], W["bkm"][:],
                                None, ALU.add)

        # mk padded (pixel QK lhsT)
        for mc in range(2):
            pm = mm_ps.tile([P, LC], F32, tag="mm")
            for kc in range(2):
                nc.tensor.matmul(pm[:, 0:P], W["wkmp"][:, kc, mc, :],
                                 m1T[:, kc, :], start=(kc == 0), stop=(kc == 1))
            nc.vector.tensor_scalar(mkpT[:, mc, :], pm[:, 0:P],
                                    W["bkmp"][:, mc:mc + 1], None, ALU.add)

        # mv for v_cat -> vaug chunk 32
        pm = mm_ps.tile([P, LC], F32, tag="mm")
        for kc in range(2):
            nc.tensor.matmul(pm[:, 0:TV], m1T[:, kc, :], W["wvm_c"][:, kc, :],
                             start=(kc == 0), stop=(kc == 1))
        nc.vector.tensor_tensor(
            vaug[:, 32].rearrange("p (pr c) -> p pr c", c=65)[:, :, 0:64],
            pm[:, 0:TV].rearrange("p (pr c) -> p pr c", c=64),
            W["bvm_c_bc"][:].rearrange("p (pr c) -> p pr c", c=64), ALU.add)

        # mv for pixel AV (scaled); plus zero-padded per-head variant
        pm = mm_ps.tile([P, LC], F32, tag="mm")
        for kc in range(2):
            nc.tensor.matmul(pm[:, 0:TV], m1T[:, kc, :], W["wvm_p"][:, kc, :],
                             start=(kc == 0), stop=(kc == 1))
        nc.vector.tensor_tensor(mvpx[:], pm[:, 0:TV], W["bvm_p_bc"][:], ALU.add)
        nc.vector.memset(mvpx_pad[:].bitcast(F32), 0.0)
        for h in range(H):
            g_, i_ = divmod(h, 4)
            nc.vector.tensor_copy(
                mvpx_pad[:, g_, i_, 32 * i_:32 * i_ + 32],
                mvpx[:, 32 * h:32 * h + 32])

        # colsum of mvpx (pixel AV bias) + b_pret
        pm = mm_ps.tile([P, LC], F32, tag="mm")
        for c_ in range(2):
            nc.tensor.matmul(pm[:, 2 * c_:2 * c_ + 2],
                             mvpx[:, P * c_:P * (c_ + 1)],
                             ones_r[:], start=True, stop=True)
        nc.vector.tensor_tensor(biaspret[:], pm[:, 0:4:2], W["b_pret_col"][:],
                                ALU.add)

        # ================= PHASE B: pixel pipeline =================
        # Software-pipelined emission: front(c+1) is emitted before tail(c) so
        # the scheduler can fill attention-phase gaps with next-chunk work.
        def _front(c):
            l0 = c * LC
            pst = tin_ps.tile([P, LC], F32, tag="tin", name="pst")
            for j in range(4):
                if c < 2:
                    xnat = xp_pre[:, 4 * c + j, :]
                else:
                    xt_ = trans3.tile([P, P], F32, tag="xnat", name="xnat")
                    nc.sync.dma_start(xt_[:], xp_d[l0 + P * j:l0 + P * (j + 1), :])
                    xnat = xt_[:]
                nc.tensor.transpose(pst[:, P * j:P * (j + 1)], xnat, ident[:])
            nc.vector.tensor_copy(xpT[:, l0:l0 + LC], pst[:])

            # conv1 -> P1T [2][128, 512]
            p1T = deep3.tile([P, 2, LC], F32R, tag="p1T", name="p1T")
            for mc in range(2):
                pm = tin_ps.tile([P, LC], F32, tag="tin", name="pm")
                nc.tensor.matmul(pm[:], W["w1"][:, mc, :],
                                 xpT[:, l0:l0 + LC],
                                 start=True, stop=True)
                nc.scalar.activation(p1T[:, mc, :], pm[:], AF.Relu,
                                     bias=W["b1"][:, mc:mc + 1])

            # qkv projections
            pqTp = deep3.tile([P, 2, LC], F32R, tag="pqTp", name="pqTp")
            for mc in range(2):
                pm = mm_ps.tile([P, LC], F32, tag="mm", name="pm")
                for kc in range(2):
                    nc.tensor.matmul(pm[:], W["wq"][:, kc, mc, :], p1T[:, kc, :],
                                     start=(kc == 0), stop=(kc == 1))
                nc.vector.tensor_scalar(pqTp[:, mc, :], pm[:],
                                        W["bq"][:, mc:mc + 1], None, ALU.add)
            pm = mm_ps.tile([P, LC], F32, tag="mm", name="pm")
            for kc in range(2):
                nc.tensor.matmul(pm[:], W["wk"][:, kc, :], p1T[:, kc, :],
                                 start=(kc == 0), stop=(kc == 1))
            nc.vector.tensor_scalar(kcatT[:, l0:l0 + LC], pm[:], W["bk"][:],
                                    None, ALU.add)
            for lt in range(4):
                pm = mm_ps.tile([P, LC], F32, tag="mm", name="pm")
                for kc in range(2):
                    nc.tensor.matmul(pm[:, 0:TV],
                                     p1T[:, kc, P * lt:P * (lt + 1)],
                                     W["wv_p"][:, kc, :],
                                     start=(kc == 0), stop=(kc == 1))
                nc.vector.tensor_tensor(
                    vaug[:, 4 * c + lt].rearrange("p (pr x) -> p pr x", x=65)
                    [:, :, 0:64],
                    pm[:, 0:TV].rearrange("p (pr x) -> p pr x", x=64),
                    W["bv_p_bc"][:].rearrange("p (pr x) -> p pr x", x=64),
                    ALU.add)
            return pqTp

        def _tail(c, pqTp):
            l0 = c * LC
            # pixel QK (row-packed strips, 2 heads per round) + tanh,
            # then AV per 4-head group (4 accumulating zero-padded MMs)
            pretT = trans.tile([P, 2, LC], F32R, tag="pretT", name="pretT")
            for g_ in range(2):
                pattn = trans.tile([P, 4, LC], F32R, tag="pattn", name="pattn")
                for r in range(2):
                    pq_ = qk_ps.tile([P, 2 * LC], F32, tag="qk", name="pq_")
                    for i in range(2):
                        h = 4 * g_ + 2 * r + i
                        pos = 32 * (h % 4)
                        nc.tensor.matmul(pq_[:, LC * i:LC * (i + 1)],
                                         mkpT[pos:pos + 32, h // 4, :],
                                         pqTp[pos:pos + 32, h // 4, :],
                                         start=True, stop=True,
                                         tile_position=(pos, 0))
                    nc.scalar.activation(
                        pattn[:, 2 * r:2 * r + 2, :]
                        .rearrange("p a b -> p (a b)"),
                        pq_[:], AF.Tanh, scale=0.5)
                pm = mm_ps.tile([P, LC], F32, tag="mm", name="pm")
                for i in range(4):
                    nc.tensor.matmul(pm[:], mvpx_pad[:, g_, i, :],
                                     pattn[:, i, :],
                                     start=(i == 0), stop=(i == 3))
                nc.vector.tensor_scalar(pretT[:, g_, :], pm[:],
                                        biaspret[:, g_:g_ + 1], 0.0,
                                        ALU.add, ALU.max)

            # conv3 + residual + relu (feature-major), then transpose out
            pm = mm_ps.tile([P, LC], F32, tag="mm", name="pm")
            for kc in range(2):
                nc.tensor.matmul(pm[:], W["w3p"][:, kc, :], pretT[:, kc, :],
                                 start=(kc == 0), stop=(kc == 1))
            poutT = trans.tile([P, LC], F32, tag="poutT", name="poutT")
            nc.vector.tensor_tensor(poutT[:], pm[:], xpT[:, l0:l0 + LC], ALU.add)
            nc.vector.tensor_scalar(poutT[:], poutT[:], W["b3p"][:], 0.0,
                                    ALU.add, ALU.max)
            pst2 = mm_ps.tile([P, LC], F32, tag="mm", name="pst2")
            for j in range(4):
                nc.tensor.transpose(pst2[:, P * j:P * (j + 1)],
                                    poutT[:, P * j:P * (j + 1)], ident[:])
            pout = trans.tile([P, LC], F32, tag="poutT", name="pout")
            nc.vector.tensor_copy(pout[:], pst2[:])
            for j in range(4):
                nc.sync.dma_start(ypix_d[l0 + P * j:l0 + P * (j + 1), :],
                                  pout[:, P * j:P * (j + 1)])

        # ---- memory attention m-tile (phase C body), interleavable ----
        avp_box = {}

        def _memtile(t):
            pqm = qk_ps.tile([P, 2 * LC], F32, tag="qk", name="pqm")
            for u in range(2):
                nc.tensor.matmul(pqm[:, LC * u:LC * (u + 1)],
                                 kcatT[:, P * t:P * (t + 1)],
                                 mq_bd[:, LC * u:LC * (u + 1)],
                                 start=True, stop=True)
            probs = trans3.tile([P, 2 * LC], F32R, tag="probs", name="probs")
            nc.scalar.activation(probs[:], pqm[:], AF.Exp)
            for pr in range(4):
                nc.tensor.matmul(
                    avp_box["a"][pr // 2]
                    [0:65, 256 * (pr % 2):256 * (pr % 2) + 256],
                    vaug[:, t, 65 * pr:65 * pr + 65],
                    probs[:, 256 * pr:256 * (pr + 1)],
                    start=(t == 0), stop=(t == MT - 1))

        pend = {}
        done_mt = 0
        for c in range(NCHUNK + 1):
            if c < NCHUNK:
                pend[c] = _front(c)
            if c >= 1:
                _tail(c - 1, pend.pop(c - 1))
            # interleave early memory-attention m-tiles under the last pixel
            # chunks (m-tile t needs only pixel chunk t//4's k/v outputs)
            if c >= 4:
                if "a" not in avp_box:
                    avp_box["a"] = [
                        mm_ps.tile([P, LC], F32, tag="mm", name=f"av{q}")
                        for q in range(2)]
                lim = min(MT, 9 * (c - 3))
                while done_mt < lim:
                    _memtile(done_mt)
                    done_mt += 1

        # ================= PHASE C: remaining memory attention ==========
        while done_mt < MT:
            _memtile(done_mt)
            done_mt += 1
        avp = avp_box["a"]

        # ================= PHASE D: memory tail =================
        # denominator rows: one pair at a time through a small row-64 buffer
        dsb = persist.tile([P, 512], F32, tag="dsb")
        for pr in range(4):
            q, s_ = divmod(pr, 2)
            col = 256 * (pr % 2)
            nc.vector.tensor_copy(dsb[64:65, col:col + 256],
                                  avp[q][64:65, 256 * s_:256 * (s_ + 1)])
            psd = qk_ps.tile([P, 2 * LC], F32, tag="qk")
            nc.tensor.matmul(psd[0:64, 0:256], ones64[64:65, :],
                             dsb[64:65, col:col + 256],
                             start=True, stop=True, tile_position=(64, 0))
            recip = trans3.tile([64, 256], F32, tag="recip")
            nc.vector.reciprocal(recip[:], psd[0:64, 0:256])
            ro = 64 * s_
            nc.vector.tensor_tensor(retT[ro:ro + 32, q, :],
                                    avp[q][0:32, 256 * s_:256 * s_ + P],
                                    recip[0:32, 0:P], ALU.mult)
            nc.vector.tensor_tensor(retT[ro + 32:ro + 64, q, :],
                                    avp[q][32:64, 256 * s_ + P:256 * (s_ + 1)],
                                    recip[32:64, P:2 * P], ALU.mult)
        for q in range(2):
            nc.vector.tensor_scalar(retT[:, q, :], retT[:, q, :],
                                    W["b_mret_col"][:, q:q + 1], 0.0,
                                    ALU.add, ALU.max)

        if dbg:
            for q in range(2):
                dd = trans.tile([P, LC], F32, tag="poutT", name="dd")[:, 0:P]
                nc.vector.tensor_copy(dd[:], retT[:, q, :])
                nc.sync.dma_start(dbg["d_ret"][:, q, :], dd[:])
                da = trans.tile([P, LC], F32, tag="poutT", name="da")
                nc.vector.tensor_copy(da[:], avp[q][:])
                nc.sync.dma_start(dbg["d_av"][q], da[:])
                dm1 = trans.tile([P, LC], F32, tag="poutT", name="dm1")[:, 0:P]
                nc.vector.tensor_copy(dm1[:], m1T[:, q, :])
                nc.sync.dma_start(dbg["d_m1"][:, q, :], dm1[:])
            nc.sync.dma_start(dbg["d_dsb"][:, 0:256], dsb[:, 0:256])

        # mem conv3 feature-major (+ residual + relu) -> moT directly
        pm = mm_ps.tile([P, LC], F32, tag="mm")
        for kc in range(2):
            nc.tensor.matmul(pm[:, 0:P], W["wm3"][:, kc, :], retT[:, kc, :],
                             start=(kc == 0), stop=(kc == 1))
        moT = trans.tile([P, P], F32, tag="moT")
        nc.vector.tensor_tensor(moT[:], pm[:, 0:P], xm_b3T[:], ALU.add)
        nc.vector.tensor_scalar(moT[:], moT[:], 0.0, None, ALU.max)
        moTr = trans.tile([P, P], F32R, tag="moTr")
        nc.vector.tensor_copy(moTr[:], moT[:])

        if dbg:
            dmo = trans.tile([P, LC], F32, tag="poutT", name="dmo")[:, 0:P]
            nc.vector.tensor_copy(dmo[:], mo[:])
            nc.sync.dma_start(dbg["d_mo"][:], dmo[:])

        # ffn1 -> ffnT [128, 16, 128]
        ffnT = persist.tile([P, 16, P], F32R, tag="ffnT")
        for g_ in range(4):
            pm = mm_ps.tile([P, LC], F32, tag="mm")
            for s_ in range(4):
                mc = 4 * g_ + s_
                nc.tensor.matmul(pm[:, P * s_:P * (s_ + 1)],
                                 W["wf1"][:, mc, :], moTr[:],
                                 start=True, stop=True)
            for s_ in range(4):
                mc = 4 * g_ + s_
                nc.scalar.activation(ffnT[:, mc, :],
                                     pm[:, P * s_:P * (s_ + 1)], AF.Relu,
                                     bias=W["bf1"][:, mc:mc + 1])
        if dbg:
            for mc in range(16):
                df = trans.tile([P, LC], F32, tag="poutT", name="df")[:, 0:P]
                nc.vector.tensor_copy(df[:], ffnT[:, mc, :])
                nc.sync.dma_start(dbg["d_ffn"][:, mc, :], df[:])

        # ffn2 (+ residual + relu) -> transpose -> ymem
        # two independent accumulators so the chain overlaps ffn1 production
        pma = mm_ps.tile([P, LC], F32, tag="mm", name="pma")
        pmb = mm_ps.tile([P, LC], F32, tag="mm", name="pmb")
        for kc in range(8):
            nc.tensor.matmul(pma[:, 0:P], W["wf2"][:, kc, :], ffnT[:, kc, :],
                             start=(kc == 0), stop=(kc == 7))
        for kc in range(8, 16):
            nc.tensor.matmul(pmb[:, 0:P], W["wf2"][:, kc, :], ffnT[:, kc, :],
                             start=(kc == 8), stop=(kc == 15))
        mo2T = trans.tile([P, P], F32, tag="mo2T")
        nc.vector.tensor_tensor(mo2T[:], pma[:, 0:P], moT[:], ALU.add)
        nc.vector.tensor_tensor(mo2T[:], pmb[:, 0:P], mo2T[:], ALU.add)
        nc.vector.tensor_scalar(mo2T[:], mo2T[:], W["bf2"][:], 0.0,
                                ALU.add, ALU.max)
        pst = mm_ps.tile([P, LC], F32, tag="mm")
        nc.tensor.transpose(pst[:, 0:P], mo2T[:], ident[:])
        mo2 = trans.tile([P, P], F32, tag="mo2")
        nc.vector.tensor_copy(mo2[:], pst[:, 0:P])
        nc.sync.dma_start(ymem_d[:], mo2[:])


def kernel(**inputs):
    if "nc" not in _cached:
        _cached["nc"] = _build_program()
    nc = _cached["nc"]
    d = _prep_host(inputs)
    pix = np.ascontiguousarray(np.asarray(inputs["pixel_input"], np.float32))
    mem = np.ascontiguousarray(np.asarray(inputs["memory_input"], np.float32))
    in_maps = []
    for b in range(B):
        m = {"xp": pix[b], "xm": mem[b]}
        m.update(d)
        in_maps.append(m)
    res = run_bass_kernel_spmd(nc, in_maps, core_ids=list(range(B)))
    pix_out = np.stack([res.results[b]["ypix"] for b in range(B)])
    mem_out = np.stack([res.results[b]["ymem"] for b in range(B)])
    return pix_out, mem_out
